# revision 1
# baseline (speedup 1.0000x reference)
"""Trainium2 Bass kernel for nn_MoEAugmentedActor (moe_routing).

Strategy (pure data parallel, 8 cores, batch-sharded):
  - Host prepares a feature-major fp16 view xT of the needed x columns
    (579 of 975), with ones-rows baked in so L1 biases ride the matmul.
  - On-chip everything is feature-major: [features(part), batch(free)],
    batch tiled at 512 columns.
  - ELU(y) is computed as  elu(y)+1 = max(y+1, min(e^y, 1)):
      psum holds y+1 (bias rows are b+1), ACT does t = Exp(psum-1),
      DVE scalar_tensor_tensor does u = (t min 1) max psum  in one pass.
    The +1 shift is absorbed into the next layer's bias on the host
    (b' = b - colsum(W)).
  - Gate logits are replicated into 32-aligned 29-row blocks by an
    expanded gate-L2 matmul so per-expert softmax weights can be read
    as legal SBUF slices; softmax runs without max-subtraction
    (logits are tiny).  Blend: se_e = (pacts_e + b3'_e) * exp(gl_e)
    via one scalar_tensor_tensor per expert, summed, then multiplied
    by the broadcast reciprocal of the partition-summed exp.
  - Device writes out feature-major [29, n]; host transposes back.
"""

import os
import sys

for _p in ("/opt/trn_rl_repo", "/root/.axon_site/_ro/trn_rl_repo"):
    if os.path.isdir(_p) and _p not in sys.path:
        sys.path.insert(0, _p)

import numpy as np

# ----------------------------------------------------------------- constants
N_FULL = 131072
N_CORES = 8
N_CORE = N_FULL // N_CORES  # 16384
TILE = 512  # batch columns per tile

OBS_TERM_DIMS = (3, 3, 3, 3, 29, 29, 29, 96)
HISTORY_LEN = 5
_OFFS = [0]
for _d in OBS_TERM_DIMS[:-1]:
    _OFFS.append(_OFFS[-1] + _d * HISTORY_LEN)

# vae_hist column order: frame i in 0..4, terms 1..6, dims within term
VAE_COLS = [
    _OFFS[t] + i * OBS_TERM_DIMS[t] + j
    for i in range(HISTORY_LEN)
    for t in range(1, 7)
    for j in range(OBS_TERM_DIMS[t])
]  # 480
OT_COLS = [
    _OFFS[t] + 4 * OBS_TERM_DIMS[t] + j for t in range(7) for j in range(OBS_TERM_DIMS[t])
]  # 99
ELEV_COLS = list(range(_OFFS[7] + 4 * 96, _OFFS[7] + 5 * 96))  # 96

XT_ROWS = 784  # 6 blocks of 128 + zeros/ones tail
WCOLS = 4224


def _w_offsets():
    off = {}
    c = 0

    def take(name, n):
        nonlocal c
        off[name] = c
        c += n

    take("w1", 4 * 256)       # 4 chunks x [K,256]
    take("wzv", 2 * 35)       # 2 chunks x [128,35]  ([Wv|Wz] order)
    take("ae1", 64)           # [97,64]
    take("ae2", 32)           # [64,32]
    take("g1", 64)            # [33,64] stored at partitions 64..96
    take("g2", 5)             # [64,5]
    take("g2r1", 128)         # [64,128] replicated gate cols, experts 0..3
    take("g2r2", 29)          # [64,29]  replicated gate col, expert 4
    take("e1a", 5 * 128)      # [99,128] x5
    take("e1b", 5 * 128)      # [97,128] x5
    take("c2", 5 * 128)       # [1,128] x5
    take("e2", 5 * 128)       # [128,128] x5
    take("e3", 5 * 32)        # [128,32] x5 (padded to 32)
    take("ones5", 1)          # [5,1]
    take("msum", 29)          # [128,29] 0/1 block-sum matrix
    take("i29", 29)           # [29,29] identity
    assert c <= WCOLS, c
    return off


WOFF = _w_offsets()

# bpack columns
BC_ZV = 0      # rows 0..34:  [bv|bz]' adjusted
BC_ZE = 1      # rows 0..31:  ae_b2'
BC_G2 = 2      # rows 0..4:   gate_b2'
BC_NEG1 = 3    # all rows: -1.0
BC_G2R = 4     # rows 32e+k (e<4,k<29): gate_b2'_e   (replicated-logit bias)
BC_G2R4 = 5    # rows 0..28: gate_b2'_4
BC_B3 = 6      # rows 32e+k (e<4,k<29): b3'_e[k]
BC_B34 = 7     # rows 0..28: b3'_4
BC_C2 = 8      # cols 8..12:  expert-L2 bias c2_e (rows 0..127)
BC_C2P1 = 13   # cols 13..17: c2_e + 1
NBCOLS = 18


# ----------------------------------------------------------------- device IR

def build_program(n_rows=N_CORE, num_devices=N_CORES):
    """Build + compile the per-core Bass program. Returns nc."""
    import concourse.bass as bass
    import concourse.mybir as mybir
    from concourse import bacc
    from concourse.tile import TileContext

    fp16 = mybir.dt.float16
    fp32 = mybir.dt.float32
    AF = mybir.ActivationFunctionType
    OP = mybir.AluOpType

    n_tiles = n_rows // TILE
    assert n_rows % TILE == 0

    nc = bacc.Bacc("TRN2", target_bir_lowering=False, debug=False,
                   num_devices=num_devices)

    xT = nc.dram_tensor("xT", (XT_ROWS, n_rows), fp16, kind="ExternalInput").ap()
    wpack = nc.dram_tensor("wpack", (128, WCOLS), fp16, kind="ExternalInput").ap()
    bpack = nc.dram_tensor("bpack", (128, NBCOLS), fp32, kind="ExternalInput").ap()
    out_fm = nc.dram_tensor("out_fm", (29, n_rows), fp32, kind="ExternalOutput").ap()

    with TileContext(nc) as tc:
        with (
            tc.tile_pool(name="const", bufs=1) as constp,
            tc.tile_pool(name="xio", bufs=4) as xio,
            tc.tile_pool(name="inp", bufs=4) as inpp,
            tc.tile_pool(name="uh", bufs=6) as uhp,
            tc.tile_pool(name="small", bufs=3) as smallp,
            tc.tile_pool(name="texp", bufs=10) as texpp,
            tc.tile_pool(name="u12", bufs=8) as u12p,
            tc.tile_pool(name="blend", bufs=4) as blendp,
            tc.tile_pool(name="pe", bufs=2, space="PSUM") as pep,
            tc.tile_pool(name="pmain", bufs=3, space="PSUM") as pmainp,
            tc.tile_pool(name="pacts", bufs=1, space="PSUM") as pactsp,
        ):
            # ---- persistent constants
            wsb = constp.tile([128, WCOLS], fp16, tag="wsb")
            nc.sync.dma_start(out=wsb, in_=wpack)
            bsb = constp.tile([128, NBCOLS], fp32, tag="bsb")
            nc.sync.dma_start(out=bsb, in_=bpack)
            ones_all = constp.tile([128, TILE], fp16, tag="ones_all")
            nc.vector.memset(ones_all, 1.0)

            xT_blk = xT[0:640].rearrange("(b p) n -> p b n", p=128)  # [128, 5, n]

            def w(name, k, m, idx=0, msz=None, prow=0):
                base = WOFF[name] + idx * m
                return wsb[prow:prow + k, base:base + (msz if msz is not None else m)]

            def elu(pool, tag, psum, m, fd=TILE):
                """psum[0:m, 0:fd] holds y+1 -> u = elu(y)+1 fp16."""
                t = texpp.tile([128, 2 * TILE], fp16, tag="texp")
                nc.scalar.activation(t[0:m, 0:fd], psum[0:m, 0:fd], AF.Exp,
                                     bias=bsb[0:m, BC_NEG1:BC_NEG1 + 1], scale=1.0)
                u = pool.tile([128, 2 * TILE], fp16, tag=tag)
                nc.vector.scalar_tensor_tensor(
                    out=u[0:m, 0:fd], in0=t[0:m, 0:fd], scalar=1.0,
                    in1=psum[0:m, 0:fd], op0=OP.min, op1=OP.max)
                return u

            pending_blend = [None]
            for it in range(n_tiles):
                n0 = it * TILE
                # ---- loads
                xsb = xio.tile([128, 5, TILE], fp16, tag="xsb")
                nc.sync.dma_start(out=xsb, in_=xT_blk[:, 0:5, n0:n0 + TILE])
                inpA = inpp.tile([128, TILE], fp16, tag="inpA")
                nc.sync.dma_start(out=inpA, in_=xT[640:768, n0:n0 + TILE])
                inpB = inpp.tile([128, TILE], fp16, tag="inpB")
                nc.sync.dma_start(out=inpB[32:64], in_=xT[739:771, n0:n0 + TILE])
                nc.sync.dma_start(out=inpB[97:128], in_=xT[739:770, n0:n0 + TILE])
                nc.sync.dma_start(out=inpB[96:97], in_=xT[771:772, n0:n0 + TILE])

                # early expert-L1 A-chunks: only need inpA (o_t DMA)
                pe1a = pep.tile([128, 2 * TILE], fp32, tag="pe")
                pe1b = pep.tile([128, 2 * TILE], fp32, tag="pe")
                for pe1_, pair_ in ((pe1a, (0, 1)), (pe1b, (2, 3))):
                    for j_, e_ in enumerate(pair_):
                        nc.tensor.matmul(pe1_[:, j_ * TILE:(j_ + 1) * TILE],
                                         lhsT=w("e1a", 128, 128, e_),
                                         rhs=inpA, start=True, stop=False)

                if pending_blend[0] is not None:
                    pending_blend[0]()
                    pending_blend[0] = None

                # ---- VAE L1: two 128-halves, separate psum tiles
                u_hs = []
                for half in (0, 1):
                    ph = pmainp.tile([128, TILE], fp32, tag="pmain")
                    for c in range(4):
                        nc.tensor.matmul(
                            ph,
                            lhsT=wsb[0:128, WOFF["w1"] + c * 256 + half * 128:
                                     WOFF["w1"] + c * 256 + half * 128 + 128],
                            rhs=xsb[0:128, c, :],
                            start=(c == 0), stop=(c == 3))
                    u_hs.append(elu(uhp, "uh", ph, 128))
                u_h0, u_h1 = u_hs

                # ---- VAE L2 -> [v_pred(3) | z_H(32)] into inpB[0:35]
                pz = pmainp.tile([128, TILE], fp32, tag="pmain")
                nc.tensor.matmul(pz[0:35], lhsT=w("wzv", 128, 35, 0),
                                 rhs=u_h0[:, 0:TILE], start=True, stop=False)
                nc.tensor.matmul(pz[0:35], lhsT=w("wzv", 128, 35, 1),
                                 rhs=u_h1[:, 0:TILE], start=False, stop=True)
                nc.scalar.activation(inpB[0:35], pz[0:35], AF.Identity,
                                     bias=bsb[0:35, BC_ZV:BC_ZV + 1], scale=1.0)

                # ---- AE: ha -> z_E(32) into inpB[64:96]
                pa = pmainp.tile([128, TILE], fp32, tag="pmain")
                nc.tensor.matmul(pa[0:64], lhsT=w("ae1", 128, 64), rhs=xsb[0:128, 4, :],
                                 start=True, stop=True)
                u_a = elu(uhp, "uh", pa, 64)
                pzE = pmainp.tile([128, TILE], fp32, tag="pmain")
                nc.tensor.matmul(pzE[0:32], lhsT=w("ae2", 64, 32), rhs=u_a[0:64, 0:TILE],
                                 start=True, stop=True)
                nc.scalar.activation(inpB[64:96], pzE[0:32], AF.Identity,
                                     bias=bsb[0:32, BC_ZE:BC_ZE + 1], scale=1.0)

                # ---- experts: pairs (0,1)+(2,3) interleaved; e4 single lane
                pacts0 = pactsp.tile([128, TILE], fp32, tag="pacts")

                def l1b_mms(pair, pe1):
                    for j, e in enumerate(pair):
                        sl = slice(j * TILE, (j + 1) * TILE)
                        nc.tensor.matmul(pe1[:, sl], lhsT=w("e1b", 128, 128, e),
                                         rhs=inpB, start=False, stop=True)

                def l2_mms(pair, u1):
                    pe2 = pep.tile([128, 2 * TILE], fp32, tag="pe")
                    for j, e in enumerate(pair):
                        sl = slice(j * TILE, (j + 1) * TILE)
                        nc.tensor.matmul(pe2[:, sl], lhsT=w("e2", 128, 128, e),
                                         rhs=u1[:, sl], start=True, stop=True)
                    return pe2

                def l2_elu(pair, pe2):
                    fd = len(pair) * TILE
                    t2 = texpp.tile([128, 2 * TILE], fp16, tag="texp")
                    for j, e in enumerate(pair):
                        sl = slice(j * TILE, (j + 1) * TILE)
                        nc.scalar.activation(t2[:, sl], pe2[:, sl], AF.Exp,
                                             bias=bsb[0:128, BC_C2 + e:BC_C2 + e + 1],
                                             scale=1.0)
                    s2 = texpp.tile([128, 2 * TILE], fp16, tag="s2")
                    nc.vector.tensor_scalar(out=s2[:, 0:fd], in0=t2[:, 0:fd],
                                            scalar1=1.0, scalar2=None, op0=OP.min)
                    u2 = u12p.tile([128, 2 * TILE], fp16, tag="u12")
                    for j, e in enumerate(pair):
                        sl = slice(j * TILE, (j + 1) * TILE)
                        nc.vector.scalar_tensor_tensor(
                            out=u2[:, sl], in0=pe2[:, sl],
                            scalar=bsb[0:128, BC_C2P1 + e:BC_C2P1 + e + 1],
                            in1=s2[:, sl], op0=OP.add, op1=OP.max)
                    return u2

                def l3_mms(pair, u2):
                    for j, e in enumerate(pair):
                        sl = slice(j * TILE, (j + 1) * TILE)
                        if e < 4:
                            nc.tensor.matmul(pacts0[32 * e:32 * e + 32],
                                             lhsT=w("e3", 128, 32, e), rhs=u2[:, sl],
                                             start=True, stop=True,
                                             tile_position=(0, 32 * e))
                        else:
                            pacts1 = pmainp.tile([128, TILE], fp32, tag="pmain")
                            nc.tensor.matmul(pacts1[0:29],
                                             lhsT=w("e3", 128, 32, e, msz=29),
                                             rhs=u2[:, sl], start=True, stop=True)
                            globals_pacts1[0] = pacts1
                    return globals_pacts1[0] if pair == (4,) else None

                globals_pacts1 = [None]
                pA, pB = (0, 1), (2, 3)
                l1b_mms(pA, pe1a)
                l1b_mms(pB, pe1b)
                u1a = elu(u12p, "u12", pe1a, 128, 2 * TILE)
                pe2a = l2_mms(pA, u1a)
                u1b = elu(u12p, "u12", pe1b, 128, 2 * TILE)
                pe2b = l2_mms(pB, u1b)
                u2a = l2_elu(pA, pe2a)
                l3_mms(pA, u2a)

                # ---- gate chain (mid-expert: its PE waits overlap pair-B work)
                pg = pmainp.tile([128, TILE], fp32, tag="pmain")
                nc.tensor.matmul(pg[0:64], lhsT=w("g1", 33, 64, prow=64),
                                 rhs=inpB[64:97], start=True, stop=True)
                u_g = elu(uhp, "uh", pg, 64)
                pgl = pmainp.tile([128, TILE], fp32, tag="pmain")
                nc.tensor.matmul(pgl[0:5], lhsT=w("g2", 64, 5), rhs=u_g[0:64, 0:TILE],
                                 start=True, stop=True)
                t_gate = smallp.tile([5, TILE], fp16, tag="tgate")
                nc.scalar.activation(t_gate, pgl[0:5], AF.Exp,
                                     bias=bsb[0:5, BC_G2:BC_G2 + 1], scale=1.0)
                pglR = pmainp.tile([128, TILE], fp32, tag="pmain")
                nc.tensor.matmul(pglR, lhsT=w("g2r1", 64, 128), rhs=u_g[0:64, 0:TILE],
                                 start=True, stop=True)
                eg = smallp.tile([128, TILE], fp16, tag="eg")
                nc.scalar.activation(eg, pglR, AF.Exp,
                                     bias=bsb[0:128, BC_G2R:BC_G2R + 1], scale=1.0)
                pglR4 = pmainp.tile([128, TILE], fp32, tag="pmain")
                nc.tensor.matmul(pglR4[0:29], lhsT=w("g2r2", 64, 29),
                                 rhs=u_g[0:64, 0:TILE], start=True, stop=True)
                eg4 = smallp.tile([29, TILE], fp16, tag="eg4")
                nc.scalar.activation(eg4, pglR4[0:29], AF.Exp,
                                     bias=bsb[0:29, BC_G2R4:BC_G2R4 + 1], scale=1.0)
                pd = pmainp.tile([128, TILE], fp32, tag="pmain")
                nc.tensor.matmul(pd[0:1], lhsT=w("ones5", 5, 1), rhs=t_gate,
                                 start=True, stop=True)
                rd = smallp.tile([1, TILE], fp32, tag="rd")
                nc.vector.reciprocal_approx_fast(rd, pd[0:1])
                rb29 = smallp.tile([29, TILE], fp32, tag="rb29")
                nc.gpsimd.partition_broadcast(rb29, rd, channels=29)

                u2b = l2_elu(pB, pe2b)
                l3_mms(pB, u2b)
                # expert 4 single lane
                pe14 = pmainp.tile([128, TILE], fp32, tag="pmain")
                nc.tensor.matmul(pe14, lhsT=w("e1a", 128, 128, 4),
                                 rhs=inpA, start=True, stop=False)
                nc.tensor.matmul(pe14, lhsT=w("e1b", 128, 128, 4),
                                 rhs=inpB, start=False, stop=True)
                u14 = elu(u12p, "u12", pe14, 128)
                pe24 = pmainp.tile([128, TILE], fp32, tag="pmain")
                nc.tensor.matmul(pe24, lhsT=w("e2", 128, 128, 4),
                                 rhs=u14[:, 0:TILE], start=True, stop=True)
                u24 = l2_elu((4,), pe24)
                pacts1 = l3_mms((4,), u24)


                def make_blend(bn0, pacts0, pacts1, eg, eg4, rb29):
                    def emit_blend():
                        # ---- blend: s_all = (pacts0 + b3') * eg covers experts 0..3
                        s_all = blendp.tile([128, TILE], fp16, tag="s_all")
                        nc.vector.scalar_tensor_tensor(
                            out=s_all, in0=pacts0, scalar=bsb[0:128, BC_B3:BC_B3 + 1],
                            in1=eg, op0=OP.add, op1=OP.mult)
                        se4 = blendp.tile([29, TILE], fp16, tag="se4")
                        nc.vector.scalar_tensor_tensor(
                            out=se4, in0=pacts1[0:29], scalar=bsb[0:29, BC_B34:BC_B34 + 1],
                            in1=eg4, op0=OP.add, op1=OP.mult)
                        pbl = pmainp.tile([128, TILE], fp32, tag="pmain")
                        nc.tensor.matmul(pbl[0:29], lhsT=w("msum", 128, 29), rhs=s_all,
                                         start=True, stop=False)
                        nc.tensor.matmul(pbl[0:29], lhsT=w("i29", 29, 29), rhs=se4,
                                         start=False, stop=True)
                        acc = blendp.tile([29, TILE], fp32, tag="acc")
                        nc.vector.tensor_mul(out=acc, in0=pbl[0:29], in1=rb29)
        
                        nc.sync.dma_start(out=out_fm[:, bn0:bn0 + TILE], in_=acc)
                    return emit_blend
                pending_blend[0] = make_blend(n0, pacts0, pacts1, eg, eg4, rb29)

            if pending_blend[0] is not None:
                pending_blend[0]()
    nc.compile()
    return nc


# ----------------------------------------------------------------- host prep

def prep_inputs(x, vae_W1, vae_b1, vae_Wz, vae_bz, vae_Wv, vae_bv,
                ae_W1, ae_b1, ae_W2, ae_b2,
                gate_W1, gate_b1, gate_W2, gate_b2,
                eW1, eb1, eW2, eb2, eW3, eb3, n_rows=N_CORE, n_cores=N_CORES):
    """Returns in_maps (list of per-core dicts)."""
    x = np.asarray(x, np.float32)
    n_total = n_rows * n_cores
    assert x.shape[0] >= n_total

    xT = np.zeros((XT_ROWS, n_total), np.float16)
    xv = x[:n_total, VAE_COLS].T.astype(np.float16)  # [480, n]
    for c in range(4):
        xT[128 * c:128 * c + 120] = xv[120 * c:120 * c + 120]
    xT[504] = 1.0
    xT[512:608] = x[:n_total, ELEV_COLS].T.astype(np.float16)
    xT[608] = 1.0
    xT[640:739] = x[:n_total, OT_COLS].T.astype(np.float16)
    xT[771] = 1.0

    wpack = np.zeros((128, WCOLS), np.float32)
    bpack = np.zeros((128, NBCOLS), np.float32)
    bpack[:, BC_NEG1] = -1.0

    def put(name, idx, arr, msz=None, prow=0):
        k, m = arr.shape
        base = WOFF[name] + idx * (msz if msz is not None else m)
        wpack[prow:prow + k, base:base + m] = arr

    W1 = np.asarray(vae_W1, np.float32)  # [480, 256] rows already in vae_hist order
    for c in range(4):
        chunk = W1[120 * c:120 * c + 120]
        if c == 3:
            chunk = np.vstack([chunk, (np.asarray(vae_b1) + 1.0)[None]])
        put("w1", c, chunk, msz=256)
    # [Wv | Wz] order so the evac lands [v_pred(3) | z_H(32)] at inpB[0:35]
    Wzv = np.concatenate([vae_Wv, vae_Wz], axis=1).astype(np.float32)  # [256,35]
    put("wzv", 0, Wzv[0:128], msz=35)
    put("wzv", 1, Wzv[128:256], msz=35)
    bpack[0:35, BC_ZV] = np.concatenate([vae_bv, vae_bz]) - Wzv.sum(0)

    put("ae1", 0, np.vstack([ae_W1, (np.asarray(ae_b1) + 1.0)[None]]))
    put("ae2", 0, np.asarray(ae_W2, np.float32))
    bpack[0:32, BC_ZE] = np.asarray(ae_b2) - np.asarray(ae_W2).sum(0)

    put("g1", 0, np.vstack([gate_W1, (np.asarray(gate_b1) + 1.0)[None]]), prow=64)
    G2 = np.asarray(gate_W2, np.float32)  # [64,5]
    put("g2", 0, G2)
    bg2 = np.asarray(gate_b2) - G2.sum(0)  # [5]
    bpack[0:5, BC_G2] = bg2
    g2r1 = np.zeros((64, 128), np.float32)
    for e in range(4):
        g2r1[:, 32 * e:32 * e + 29] = G2[:, e:e + 1]
        bpack[32 * e:32 * e + 29, BC_G2R] = bg2[e]
    put("g2r1", 0, g2r1)
    g2r2 = np.repeat(G2[:, 4:5], 29, axis=1)
    put("g2r2", 0, g2r2)
    bpack[0:29, BC_G2R4] = bg2[4]

    for e in range(5):
        W1e = np.asarray(eW1[e], np.float32)  # [166,128]
        put("e1a", e, W1e[0:99], msz=128)
        e1b = np.zeros((97, 128), np.float32)
        e1b[0:35] = W1e[99:134]      # v_pred(3) + z_H(32) weight rows
        e1b[64:96] = W1e[134:166]    # z_E rows
        e1b[96] = np.asarray(eb1[e]) + 1.0
        put("e1b", e, e1b, msz=128)
        W2e = np.asarray(eW2[e], np.float32)
        c2 = np.asarray(eb2[e]) - W2e.sum(0)
        bpack[0:128, BC_C2 + e] = c2
        bpack[0:128, BC_C2P1 + e] = c2 + 1.0
        put("e2", e, W2e, msz=128)
        W3e = np.asarray(eW3[e], np.float32)
        W3p = np.zeros((128, 32), np.float32)
        W3p[:, 0:29] = W3e
        put("e3", e, W3p, msz=32)
        b3e = np.asarray(eb3[e]) - W3e.sum(0)  # [29]
        if e < 4:
            bpack[32 * e:32 * e + 29, BC_B3] = b3e
        else:
            bpack[0:29, BC_B34] = b3e
    put("ones5", 0, np.ones((5, 1), np.float32))
    msum = np.zeros((128, 29), np.float32)
    for e in range(4):
        msum[32 * e:32 * e + 29] = np.eye(29)
    put("msum", 0, msum)
    put("i29", 0, np.eye(29, dtype=np.float32))

    wpack16 = wpack.astype(np.float16)
    in_maps = []
    for c in range(n_cores):
        in_maps.append({
            "xT": np.ascontiguousarray(xT[:, c * n_rows:(c + 1) * n_rows]),
            "wpack": wpack16,
            "bpack": bpack,
        })
    return in_maps


# ----------------------------------------------------------------- entry

_NC_CACHE = {}


def _get_program(n_rows=N_CORE, num_devices=N_CORES):
    key = (n_rows, num_devices)
    if key not in _NC_CACHE:
        _NC_CACHE[key] = build_program(n_rows, num_devices)
    return _NC_CACHE[key]


def kernel(**inputs):
    from concourse.bass_utils import run_bass_kernel_spmd

    nc = _get_program()
    in_maps = prep_inputs(**inputs)
    res = run_bass_kernel_spmd(nc, in_maps, core_ids=list(range(N_CORES)))
    out = np.empty((N_FULL, 29), np.float32)
    for c in range(N_CORES):
        out[c * N_CORE:(c + 1) * N_CORE] = res.results[c]["out_fm"].T
    return out



# revision 2
# speedup vs baseline: 1.1605x; 1.1605x over previous
"""Trainium2 Bass kernel for nn_MoEAugmentedActor (moe_routing).

Strategy (pure data parallel, 8 cores, batch-sharded):
  - Host prepares a feature-major fp16 view xT of the needed x columns
    (579 of 975), with ones-rows baked in so L1 biases ride the matmul.
  - On-chip everything is feature-major: [features(part), batch(free)],
    batch tiled at 512 columns.
  - ELU(y)+1 is computed by a single fused custom DVE op:
      u = max(min((a*(y+1) + b)^16, 1), y+1)   with a=1/16, b=15/16,
    i.e. (1+y/16)^16 ~= e^y (4 squarings).  One DVE pass per site,
    no ScalarE exp, no second select pass.  A variant (ELU8B) folds a
    per-partition bias add in and uses 3 squarings (n=8) for the
    expert-L2 sites whose bias cannot ride the matmul.
  - Three small sites (AE, gate hidden, expert-4 L1) instead use
    ScalarE exp + evac and a 2x-mode fp16 stock scalar_tensor_tensor,
    to balance load between ScalarE and VectorE.
  - Softmax over the 5 gate logits is NOT normalized on device: the
    kernel exports the fp16 logits and blends with unnormalized
    exp-weights; the host divides by the softmax denominator.
  - Device writes out feature-major [29, n] numerators + [5, n] logits;
    host normalizes and transposes back.
"""

import os
import sys

for _p in ("/opt/trn_rl_repo", "/root/.axon_site/_ro/trn_rl_repo"):
    if os.path.isdir(_p) and _p not in sys.path:
        sys.path.insert(0, _p)

import numpy as np

# ----------------------------------------------------------------- constants
N_FULL = 131072
N_CORES = 8
N_CORE = N_FULL // N_CORES  # 16384
TILE = 512  # batch columns per tile

OBS_TERM_DIMS = (3, 3, 3, 3, 29, 29, 29, 96)
HISTORY_LEN = 5
_OFFS = [0]
for _d in OBS_TERM_DIMS[:-1]:
    _OFFS.append(_OFFS[-1] + _d * HISTORY_LEN)

# vae_hist column order: frame i in 0..4, terms 1..6, dims within term
VAE_COLS = [
    _OFFS[t] + i * OBS_TERM_DIMS[t] + j
    for i in range(HISTORY_LEN)
    for t in range(1, 7)
    for j in range(OBS_TERM_DIMS[t])
]  # 480
OT_COLS = [
    _OFFS[t] + 4 * OBS_TERM_DIMS[t] + j for t in range(7) for j in range(OBS_TERM_DIMS[t])
]  # 99
ELEV_COLS = list(range(_OFFS[7] + 4 * 96, _OFFS[7] + 5 * 96))  # 96

XT_ROWS = 784  # 6 blocks of 128 + zeros/ones tail
WCOLS = 4224


def _w_offsets():
    off = {}
    c = 0

    def take(name, n):
        nonlocal c
        off[name] = c
        c += n

    take("w1", 4 * 256)       # 4 chunks x [K,256]
    take("wzv", 2 * 35)       # 2 chunks x [128,35]  ([Wv|Wz] order)
    take("ae1", 64)           # [97,64]
    take("ae2", 32)           # [64,32]
    take("g1", 64)            # [33,64] stored at partitions 64..96
    take("g2", 5)             # [64,5]
    take("g2r1", 128)         # [64,128] replicated gate cols, experts 0..3
    take("g2r2", 29)          # [64,29]  replicated gate col, expert 4
    take("e1a", 5 * 128)      # [99,128] x5
    take("e1b", 5 * 128)      # [97,128] x5
    take("c2", 5 * 128)       # [1,128] x5 (unused now)
    take("e2", 5 * 128)       # [128,128] x5
    take("e3", 5 * 32)        # [128,32] x5 (padded to 32)
    take("msum", 29)          # [128,29] 0/1 block-sum matrix
    take("i29", 29)           # [29,29] identity
    assert c <= WCOLS, c
    return off


WOFF = _w_offsets()

# bpack columns
BC_ZV = 0      # rows 0..34:  [bv|bz]' adjusted
BC_ZE = 1      # rows 0..31:  ae_b2'
BC_G2 = 2      # rows 0..4:   gate_b2'
BC_NEG1 = 3    # all rows: -1.0
BC_G2R = 4     # rows 32e+k (e<4,k<29): gate_b2'_e   (replicated-logit bias)
BC_G2R4 = 5    # rows 0..28: gate_b2'_4
BC_B3 = 6      # rows 32e+k (e<4,k<29): b3'_e[k]
BC_B34 = 7     # rows 0..28: b3'_4
BC_C2 = 8      # cols 8..12:  expert-L2 bias c2_e (rows 0..127)
BC_C2P1 = 13   # cols 13..17: c2_e + 1
NBCOLS = 18

# (1+y/n)^n exp-approx constants
EA16, EB16 = 1.0 / 16, 15.0 / 16
EA8, EB8 = 1.0 / 8, 7.0 / 8


# ------------------------------------------------------- custom DVE elu ops

_ELU_OPS = {}


def _register_elu_ops():
    """ELU16_ANT: in0 = y+1 -> max(min((in0*s0+s1)^16, 1), in0)
    ELU8B_ANT:   s = in0 + s0(per-part bias+1) -> max(min((s*s1+imm2)^8,1), s)
    """
    if _ELU_OPS:
        return _ELU_OPS
    import concourse.dve_ops as dve_ops
    from concourse.dve_spec import Spec, Src0, C0, C1, C2, One, maxx, minn, sq, lower
    from concourse.dve_ops import DveOp
    from concourse.dve_uop import DveOpSpec

    def make(name, body, ref):
        if name in dve_ops._SUB_OPCODE_FOR_NAME:
            for op in dve_ops.OPS:
                if op.name == name:
                    return op
        spec = Spec(body=body, reference=ref)
        row = max(dve_ops._SUB_OPCODE_FOR_NAME.values()) + 1
        assert row < 0x20
        shas = {}
        for ver in ("v3", "v4"):
            s = DveOpSpec(name=name, opcode=row, uops=lower(spec, ver=ver),
                          rd1_en=False)
            shas[ver] = s.sha(ver)
        op = DveOp(name, spec, subdim=False, uops_sha=shas)
        dve_ops.OPS.append(op)
        dve_ops.CUSTOM_DVE_SPECS[name] = spec
        dve_ops._SUB_OPCODE_FOR_NAME[name] = row
        return op

    b16 = maxx(minn(sq(sq(sq(sq(Src0 * C0 + C1)))), One), Src0)
    _ELU_OPS["elu16"] = make(
        "ELU16_ANT", b16,
        lambda in0, in1, s0, s1, imm2: np.maximum(
            np.minimum((in0 * s0 + s1) ** 16, 1.0), in0),
    )
    _s = Src0 + C0
    b8 = maxx(minn(sq(sq(sq(_s * C1 + C2))), One), _s)
    _ELU_OPS["elu8b"] = make(
        "ELU8B_ANT", b8,
        lambda in0, in1, s0, s1, imm2: np.maximum(
            np.minimum(((in0 + s0) * s1 + imm2) ** 8, 1.0), in0 + s0),
    )
    return _ELU_OPS


# ----------------------------------------------------------------- device IR

def build_program(n_rows=N_CORE, num_devices=N_CORES):
    """Build + compile the per-core Bass program. Returns nc."""
    import concourse.bass as bass
    import concourse.mybir as mybir
    from concourse import bacc
    from concourse.tile import TileContext

    ops = _register_elu_ops()
    ELU16, ELU8B = ops["elu16"], ops["elu8b"]

    fp16 = mybir.dt.float16
    fp32 = mybir.dt.float32
    AF = mybir.ActivationFunctionType
    OP = mybir.AluOpType

    n_tiles = n_rows // TILE
    assert n_rows % TILE == 0

    nc = bacc.Bacc("TRN2", target_bir_lowering=False, debug=False,
                   num_devices=num_devices)

    xT = nc.dram_tensor("xT", (XT_ROWS, n_rows), fp16, kind="ExternalInput").ap()
    wpack = nc.dram_tensor("wpack", (128, WCOLS), fp16, kind="ExternalInput").ap()
    bpack = nc.dram_tensor("bpack", (128, NBCOLS), fp32, kind="ExternalInput").ap()
    out_fm = nc.dram_tensor("out_fm", (29, n_rows), fp32, kind="ExternalOutput").ap()
    gl_fm = nc.dram_tensor("gl_fm", (5, n_rows), fp16, kind="ExternalOutput").ap()

    with TileContext(nc) as tc:
        with (
            tc.tile_pool(name="const", bufs=1) as constp,
            tc.tile_pool(name="xio", bufs=4) as xio,
            tc.tile_pool(name="inp", bufs=4) as inpp,
            tc.tile_pool(name="uh", bufs=3) as uhp,
            tc.tile_pool(name="small", bufs=8) as smallp,
            tc.tile_pool(name="u12", bufs=8) as u12p,
            tc.tile_pool(name="blend", bufs=6) as blendp,
            tc.tile_pool(name="pe", bufs=2, space="PSUM") as pep,
            tc.tile_pool(name="pmain", bufs=3, space="PSUM") as pmainp,
            tc.tile_pool(name="pacts", bufs=1, space="PSUM") as pactsp,
        ):
            # ---- persistent constants
            wsb = constp.tile([128, WCOLS], fp16, tag="wsb")
            nc.sync.dma_start(out=wsb, in_=wpack)
            bsb = constp.tile([128, NBCOLS], fp32, tag="bsb")
            nc.sync.dma_start(out=bsb, in_=bpack)

            xT_blk = xT[0:640].rearrange("(b p) n -> p b n", p=128)  # [128, 5, n]

            def w(name, k, m, idx=0, msz=None, prow=0):
                base = WOFF[name] + idx * m
                return wsb[prow:prow + k, base:base + (msz if msz is not None else m)]

            def bcol(col, m=128, r0=0):
                return bsb[r0:r0 + m, col:col + 1]

            def elu16(pool, tag, src, m, fd=TILE):
                """src[0:m, 0:fd] holds y+1 -> u = elu(y)+1 fp16 (one DVE op)."""
                u = pool.tile([128, 2 * TILE], fp16, tag=tag)
                nc.vector._custom_dve(ELU16, out=u[0:m, 0:fd], in0=src[0:m, 0:fd],
                                      s0=EA16, s1=EB16)
                return u

            def elu_acts(pool, tag, psum, m, biascol):
                """ScalarE-heavy elu path: exp + evac on ACT, select on DVE 2x."""
                t = smallp.tile([128, TILE], fp16, tag=tag + "_t")
                nc.scalar.activation(t[0:m], psum[0:m, 0:TILE], AF.Exp,
                                     bias=bcol(BC_NEG1, m), scale=1.0)
                s = smallp.tile([128, TILE], fp16, tag=tag + "_s")
                nc.scalar.activation(s[0:m], psum[0:m, 0:TILE], AF.Identity,
                                     bias=biascol, scale=1.0)
                u = pool.tile([128, TILE], fp16, tag=tag)
                nc.vector.scalar_tensor_tensor(
                    out=u[0:m], in0=t[0:m], scalar=1.0, in1=s[0:m],
                    op0=OP.min, op1=OP.max)
                return u

            zero_b = None  # memset-zero bias column? use bsb col of zeros
            # bpack has no all-zero column guaranteed... BC_ZV rows35+ are 0,
            # safer: make a zeros tile once
            zb = constp.tile([128, 1], fp32, tag="zb")
            nc.vector.memset(zb, 0.0)

            pending_blend = [None]
            for it in range(n_tiles):
                n0 = it * TILE
                # ---- loads
                xsb = xio.tile([128, 5, TILE], fp16, tag="xsb")
                nc.sync.dma_start(out=xsb, in_=xT_blk[:, 0:5, n0:n0 + TILE])
                inpA = inpp.tile([128, TILE], fp16, tag="inpA")
                nc.sync.dma_start(out=inpA, in_=xT[640:768, n0:n0 + TILE])
                inpB = inpp.tile([128, TILE], fp16, tag="inpB")
                nc.sync.dma_start(out=inpB[32:64], in_=xT[739:771, n0:n0 + TILE])
                nc.sync.dma_start(out=inpB[97:128], in_=xT[739:770, n0:n0 + TILE])
                nc.sync.dma_start(out=inpB[96:97], in_=xT[771:772, n0:n0 + TILE])

                # early expert-L1 A-chunks: only need inpA (o_t DMA)
                pe1a = pep.tile([128, 2 * TILE], fp32, tag="pe")
                pe1b = pep.tile([128, 2 * TILE], fp32, tag="pe")
                for pe1_, pair_ in ((pe1a, (0, 1)), (pe1b, (2, 3))):
                    for j_, e_ in enumerate(pair_):
                        nc.tensor.matmul(pe1_[:, j_ * TILE:(j_ + 1) * TILE],
                                         lhsT=w("e1a", 128, 128, e_),
                                         rhs=inpA, start=True, stop=False)

                if pending_blend[0] is not None:
                    pending_blend[0]()
                    pending_blend[0] = None

                # ---- VAE L1: two 128-halves, separate psum tiles
                u_h = uhp.tile([128, 2 * TILE], fp16, tag="uh")
                for half in (0, 1):
                    ph = pmainp.tile([128, TILE], fp32, tag="pmain")
                    for c in range(4):
                        nc.tensor.matmul(
                            ph,
                            lhsT=wsb[0:128, WOFF["w1"] + c * 256 + half * 128:
                                     WOFF["w1"] + c * 256 + half * 128 + 128],
                            rhs=xsb[0:128, c, :],
                            start=(c == 0), stop=(c == 3))
                    nc.vector._custom_dve(
                        ELU16, out=u_h[:, half * TILE:(half + 1) * TILE],
                        in0=ph[:, 0:TILE], s0=EA16, s1=EB16)

                # ---- VAE L2 -> [v_pred(3) | z_H(32)] into inpB[0:35]
                pz = pmainp.tile([128, TILE], fp32, tag="pmain")
                nc.tensor.matmul(pz[0:35], lhsT=w("wzv", 128, 35, 0),
                                 rhs=u_h[:, 0:TILE], start=True, stop=False)
                nc.tensor.matmul(pz[0:35], lhsT=w("wzv", 128, 35, 1),
                                 rhs=u_h[:, TILE:2 * TILE], start=False, stop=True)
                nc.scalar.activation(inpB[0:35], pz[0:35], AF.Identity,
                                     bias=bcol(BC_ZV, 35), scale=1.0)

                # ---- AE: ha -> z_E(32) into inpB[64:96]  (ACT-heavy elu path)
                pa = pmainp.tile([128, TILE], fp32, tag="pmain")
                nc.tensor.matmul(pa[0:64], lhsT=w("ae1", 128, 64), rhs=xsb[0:128, 4, :],
                                 start=True, stop=True)
                u_a = elu_acts(smallp, "ua", pa, 64, zb[0:64])
                pzE = pmainp.tile([128, TILE], fp32, tag="pmain")
                nc.tensor.matmul(pzE[0:32], lhsT=w("ae2", 64, 32), rhs=u_a[0:64, 0:TILE],
                                 start=True, stop=True)
                nc.scalar.activation(inpB[64:96], pzE[0:32], AF.Identity,
                                     bias=bcol(BC_ZE, 32), scale=1.0)

                # ---- experts: L1 B-chunks for 0..3
                for pe1_, pair_ in ((pe1a, (0, 1)), (pe1b, (2, 3))):
                    for j_, e_ in enumerate(pair_):
                        nc.tensor.matmul(pe1_[:, j_ * TILE:(j_ + 1) * TILE],
                                         lhsT=w("e1b", 128, 128, e_),
                                         rhs=inpB, start=False, stop=True)

                # L1 elu + L2 for pair A then pair B
                u1a = elu16(u12p, "u12", pe1a, 128, 2 * TILE)
                pe2a = pep.tile([128, 2 * TILE], fp32, tag="pe")
                for j, e in enumerate((0, 1)):
                    sl = slice(j * TILE, (j + 1) * TILE)
                    nc.tensor.matmul(pe2a[:, sl], lhsT=w("e2", 128, 128, e),
                                     rhs=u1a[:, sl], start=True, stop=True)
                u1b = elu16(u12p, "u12", pe1b, 128, 2 * TILE)
                pe2b = pep.tile([128, 2 * TILE], fp32, tag="pe")
                for j, e in enumerate((2, 3)):
                    sl = slice(j * TILE, (j + 1) * TILE)
                    nc.tensor.matmul(pe2b[:, sl], lhsT=w("e2", 128, 128, e),
                                     rhs=u1b[:, sl], start=True, stop=True)

                # ---- gate chain (overlaps expert L2/elu work)
                pg = pmainp.tile([128, TILE], fp32, tag="pmain")
                nc.tensor.matmul(pg[0:64], lhsT=w("g1", 33, 64, prow=64),
                                 rhs=inpB[64:97], start=True, stop=True)
                u_g = elu_acts(smallp, "ug", pg, 64, zb[0:64])
                pgl = pmainp.tile([128, TILE], fp32, tag="pmain")
                nc.tensor.matmul(pgl[0:5], lhsT=w("g2", 64, 5), rhs=u_g[0:64, 0:TILE],
                                 start=True, stop=True)
                glq = smallp.tile([5, TILE], fp16, tag="glq")
                nc.scalar.activation(glq, pgl[0:5], AF.Identity,
                                     bias=bcol(BC_G2, 5), scale=1.0)
                nc.sync.dma_start(out=gl_fm[:, n0:n0 + TILE], in_=glq)
                pglR = pmainp.tile([128, TILE], fp32, tag="pmain")
                nc.tensor.matmul(pglR, lhsT=w("g2r1", 64, 128), rhs=u_g[0:64, 0:TILE],
                                 start=True, stop=True)
                eg = blendp.tile([128, TILE], fp16, tag="eg")
                nc.scalar.activation(eg, pglR, AF.Exp,
                                     bias=bcol(BC_G2R), scale=1.0)
                pglR4 = pmainp.tile([128, TILE], fp32, tag="pmain")
                nc.tensor.matmul(pglR4[0:29], lhsT=w("g2r2", 64, 29),
                                 rhs=u_g[0:64, 0:TILE], start=True, stop=True)
                eg4 = blendp.tile([29, TILE], fp16, tag="eg4")
                nc.scalar.activation(eg4, pglR4[0:29], AF.Exp,
                                     bias=bcol(BC_G2R4, 29), scale=1.0)

                # ---- expert L2 elu (per-expert bias via ELU8B) + L3 col-tiled
                pacts0 = pactsp.tile([128, TILE], fp32, tag="pacts")
                u2a = u12p.tile([128, 2 * TILE], fp16, tag="u12")
                u2b = u12p.tile([128, 2 * TILE], fp16, tag="u12")
                for u2_, pe2_, pair_ in ((u2a, pe2a, (0, 1)), (u2b, pe2b, (2, 3))):
                    for j, e in enumerate(pair_):
                        sl = slice(j * TILE, (j + 1) * TILE)
                        nc.vector._custom_dve(
                            ELU8B, out=u2_[:, sl], in0=pe2_[:, sl],
                            s0=bcol(BC_C2P1 + e), s1=EA8, imm2=EB8)
                # all four L3 matmuls back-to-back -> col-tile concurrency
                for e in range(4):
                    u2_ = u2a if e < 2 else u2b
                    sl = slice((e % 2) * TILE, (e % 2 + 1) * TILE)
                    nc.tensor.matmul(pacts0[32 * e:32 * e + 32],
                                     lhsT=w("e3", 128, 32, e), rhs=u2_[:, sl],
                                     start=True, stop=True,
                                     tile_position=(0, 32 * e))

                # ---- expert 4 single lane
                pe14 = pmainp.tile([128, TILE], fp32, tag="pmain")
                nc.tensor.matmul(pe14, lhsT=w("e1a", 128, 128, 4),
                                 rhs=inpA, start=True, stop=False)
                nc.tensor.matmul(pe14, lhsT=w("e1b", 128, 128, 4),
                                 rhs=inpB, start=False, stop=True)
                u14 = elu_acts(u12p, "u14", pe14, 128, zb)
                pe24 = pmainp.tile([128, TILE], fp32, tag="pmain")
                nc.tensor.matmul(pe24, lhsT=w("e2", 128, 128, 4),
                                 rhs=u14[:, 0:TILE], start=True, stop=True)
                u24 = u12p.tile([128, TILE], fp16, tag="u24")
                nc.vector._custom_dve(ELU8B, out=u24[:, 0:TILE], in0=pe24[:, 0:TILE],
                                      s0=bcol(BC_C2P1 + 4), s1=EA8, imm2=EB8)
                pacts1 = pmainp.tile([128, TILE], fp32, tag="pmain")
                nc.tensor.matmul(pacts1[0:29], lhsT=w("e3", 128, 32, 4, msz=29),
                                 rhs=u24[:, 0:TILE], start=True, stop=True)

                def make_blend(bn0, pacts0, pacts1, eg, eg4):
                    def emit_blend():
                        # acts+bias on ACT, x gate-weight on DVE (2x fp16)
                        a_all = blendp.tile([128, TILE], fp16, tag="a_all")
                        nc.scalar.activation(a_all, pacts0, AF.Identity,
                                             bias=bcol(BC_B3), scale=1.0)
                        s_all = blendp.tile([128, TILE], fp16, tag="s_all")
                        nc.vector.tensor_tensor(out=s_all, in0=a_all, in1=eg,
                                                op=OP.mult)
                        a4 = blendp.tile([29, TILE], fp16, tag="a4")
                        nc.scalar.activation(a4, pacts1[0:29], AF.Identity,
                                             bias=bcol(BC_B34, 29), scale=1.0)
                        se4 = blendp.tile([29, TILE], fp16, tag="se4")
                        nc.vector.tensor_tensor(out=se4, in0=a4, in1=eg4,
                                                op=OP.mult)
                        pbl = pmainp.tile([128, TILE], fp32, tag="pmain")
                        nc.tensor.matmul(pbl[0:29], lhsT=w("msum", 128, 29), rhs=s_all,
                                         start=True, stop=False)
                        nc.tensor.matmul(pbl[0:29], lhsT=w("i29", 29, 29), rhs=se4,
                                         start=False, stop=True)
                        acc = blendp.tile([29, TILE], fp32, tag="acc")
                        nc.scalar.activation(acc, pbl[0:29], AF.Identity,
                                             bias=zb[0:29], scale=1.0)
                        nc.sync.dma_start(out=out_fm[:, bn0:bn0 + TILE], in_=acc)
                    return emit_blend
                pending_blend[0] = make_blend(n0, pacts0, pacts1, eg, eg4)

            if pending_blend[0] is not None:
                pending_blend[0]()
    nc.compile()
    return nc


# ----------------------------------------------------------------- host prep

def prep_inputs(x, vae_W1, vae_b1, vae_Wz, vae_bz, vae_Wv, vae_bv,
                ae_W1, ae_b1, ae_W2, ae_b2,
                gate_W1, gate_b1, gate_W2, gate_b2,
                eW1, eb1, eW2, eb2, eW3, eb3, n_rows=N_CORE, n_cores=N_CORES):
    """Returns in_maps (list of per-core dicts)."""
    x = np.asarray(x, np.float32)
    n_total = n_rows * n_cores
    assert x.shape[0] >= n_total

    xT = np.zeros((XT_ROWS, n_total), np.float16)
    xv = x[:n_total, VAE_COLS].T.astype(np.float16)  # [480, n]
    for c in range(4):
        xT[128 * c:128 * c + 120] = xv[120 * c:120 * c + 120]
    xT[504] = 1.0
    xT[512:608] = x[:n_total, ELEV_COLS].T.astype(np.float16)
    xT[608] = 1.0
    xT[640:739] = x[:n_total, OT_COLS].T.astype(np.float16)
    xT[771] = 1.0

    wpack = np.zeros((128, WCOLS), np.float32)
    bpack = np.zeros((128, NBCOLS), np.float32)
    bpack[:, BC_NEG1] = -1.0

    def put(name, idx, arr, msz=None, prow=0):
        k, m = arr.shape
        base = WOFF[name] + idx * (msz if msz is not None else m)
        wpack[prow:prow + k, base:base + m] = arr

    W1 = np.asarray(vae_W1, np.float32)  # [480, 256] rows already in vae_hist order
    for c in range(4):
        chunk = W1[120 * c:120 * c + 120]
        if c == 3:
            chunk = np.vstack([chunk, (np.asarray(vae_b1) + 1.0)[None]])
        put("w1", c, chunk, msz=256)
    # [Wv | Wz] order so the evac lands [v_pred(3) | z_H(32)] at inpB[0:35]
    Wzv = np.concatenate([vae_Wv, vae_Wz], axis=1).astype(np.float32)  # [256,35]
    put("wzv", 0, Wzv[0:128], msz=35)
    put("wzv", 1, Wzv[128:256], msz=35)
    bpack[0:35, BC_ZV] = np.concatenate([vae_bv, vae_bz]) - Wzv.sum(0)

    put("ae1", 0, np.vstack([ae_W1, (np.asarray(ae_b1) + 1.0)[None]]))
    put("ae2", 0, np.asarray(ae_W2, np.float32))
    bpack[0:32, BC_ZE] = np.asarray(ae_b2) - np.asarray(ae_W2).sum(0)

    put("g1", 0, np.vstack([gate_W1, (np.asarray(gate_b1) + 1.0)[None]]), prow=64)
    G2 = np.asarray(gate_W2, np.float32)  # [64,5]
    put("g2", 0, G2)
    bg2 = np.asarray(gate_b2) - G2.sum(0)  # [5]
    bpack[0:5, BC_G2] = bg2
    g2r1 = np.zeros((64, 128), np.float32)
    for e in range(4):
        g2r1[:, 32 * e:32 * e + 29] = G2[:, e:e + 1]
        bpack[32 * e:32 * e + 29, BC_G2R] = bg2[e]
    put("g2r1", 0, g2r1)
    g2r2 = np.repeat(G2[:, 4:5], 29, axis=1)
    put("g2r2", 0, g2r2)
    bpack[0:29, BC_G2R4] = bg2[4]

    for e in range(5):
        W1e = np.asarray(eW1[e], np.float32)  # [166,128]
        put("e1a", e, W1e[0:99], msz=128)
        e1b = np.zeros((97, 128), np.float32)
        e1b[0:35] = W1e[99:134]      # v_pred(3) + z_H(32) weight rows
        e1b[64:96] = W1e[134:166]    # z_E rows
        e1b[96] = np.asarray(eb1[e]) + 1.0
        put("e1b", e, e1b, msz=128)
        W2e = np.asarray(eW2[e], np.float32)
        c2 = np.asarray(eb2[e]) - W2e.sum(0)
        bpack[0:128, BC_C2 + e] = c2
        bpack[0:128, BC_C2P1 + e] = c2 + 1.0
        put("e2", e, W2e, msz=128)
        W3e = np.asarray(eW3[e], np.float32)
        W3p = np.zeros((128, 32), np.float32)
        W3p[:, 0:29] = W3e
        put("e3", e, W3p, msz=32)
        b3e = np.asarray(eb3[e]) - W3e.sum(0)  # [29]
        if e < 4:
            bpack[32 * e:32 * e + 29, BC_B3] = b3e
        else:
            bpack[0:29, BC_B34] = b3e
    msum = np.zeros((128, 29), np.float32)
    for e in range(4):
        msum[32 * e:32 * e + 29] = np.eye(29)
    put("msum", 0, msum)
    put("i29", 0, np.eye(29, dtype=np.float32))

    wpack16 = wpack.astype(np.float16)
    in_maps = []
    for c in range(n_cores):
        in_maps.append({
            "xT": np.ascontiguousarray(xT[:, c * n_rows:(c + 1) * n_rows]),
            "wpack": wpack16,
            "bpack": bpack,
        })
    return in_maps


# ----------------------------------------------------------------- entry

_NC_CACHE = {}


def _get_program(n_rows=N_CORE, num_devices=N_CORES):
    key = (n_rows, num_devices)
    if key not in _NC_CACHE:
        _NC_CACHE[key] = build_program(n_rows, num_devices)
    return _NC_CACHE[key]


def kernel(**inputs):
    from concourse.bass_utils import run_bass_kernel_spmd

    nc = _get_program()
    in_maps = prep_inputs(**inputs)
    res = run_bass_kernel_spmd(nc, in_maps, core_ids=list(range(N_CORES)))
    out = np.empty((N_FULL, 29), np.float32)
    for c in range(N_CORES):
        num = res.results[c]["out_fm"]            # [29, n] unnormalized
        gl = res.results[c]["gl_fm"].astype(np.float32)  # [5, n]
        den = np.exp(gl).sum(axis=0)              # softmax denominator
        out[c * N_CORE:(c + 1) * N_CORE] = (num / den[None, :]).T
    return out


# revision 5
# speedup vs baseline: 1.4728x; 1.2691x over previous
"""Trainium2 Bass kernel for nn_MoEAugmentedActor (moe_routing).

Strategy (pure data parallel, 8 cores, batch-sharded):
  - Host prepares a feature-major fp16 view xT of the needed x columns
    (579 of 975), with ones-rows baked in so L1 biases ride the matmul.
  - On-chip everything is feature-major: [features(part), batch(free)],
    batch tiled at 512 columns.
  - ELU(y)+1 is computed by a single fused custom DVE op:
      u = max(min((a*(y+1) + b)^16, 1), y+1)   with a=1/16, b=15/16,
    i.e. (1+y/16)^16 ~= e^y (4 squarings).  One DVE pass per site,
    no ScalarE exp, no second select pass.  A variant (ELU8B) folds a
    per-partition bias add in and uses 3 squarings (n=8) for the
    expert-L2 sites whose bias cannot ride the matmul.
  - Three small sites (AE, gate hidden, expert-4 L1) instead use
    ScalarE exp + evac and a 2x-mode fp16 stock scalar_tensor_tensor,
    to balance load between ScalarE and VectorE.
  - Softmax over the 5 gate logits is NOT normalized on device: the
    kernel exports the fp16 logits and blends with unnormalized
    exp-weights; the host divides by the softmax denominator.
  - Device writes out feature-major [29, n] numerators + [5, n] logits;
    host normalizes and transposes back.
"""

import os
import sys

for _p in ("/opt/trn_rl_repo", "/root/.axon_site/_ro/trn_rl_repo"):
    if os.path.isdir(_p) and _p not in sys.path:
        sys.path.insert(0, _p)

import numpy as np

# ----------------------------------------------------------------- constants
N_FULL = 131072
N_CORES = 8
N_CORE = N_FULL // N_CORES  # 16384
TILE = 512  # batch columns per tile

OBS_TERM_DIMS = (3, 3, 3, 3, 29, 29, 29, 96)
HISTORY_LEN = 5
_OFFS = [0]
for _d in OBS_TERM_DIMS[:-1]:
    _OFFS.append(_OFFS[-1] + _d * HISTORY_LEN)

# vae_hist column order: frame i in 0..4, terms 1..6, dims within term
VAE_COLS = [
    _OFFS[t] + i * OBS_TERM_DIMS[t] + j
    for i in range(HISTORY_LEN)
    for t in range(1, 7)
    for j in range(OBS_TERM_DIMS[t])
]  # 480
OT_COLS = [
    _OFFS[t] + 4 * OBS_TERM_DIMS[t] + j for t in range(7) for j in range(OBS_TERM_DIMS[t])
]  # 99
ELEV_COLS = list(range(_OFFS[7] + 4 * 96, _OFFS[7] + 5 * 96))  # 96

XT_ROWS = 784  # 6 blocks of 128 + zeros/ones tail
WCOLS = 4224


def _w_offsets():
    off = {}
    c = 0

    def take(name, n):
        nonlocal c
        off[name] = c
        c += n

    take("w1", 4 * 256)       # 4 chunks x [K,256]
    take("wzv", 2 * 35)       # 2 chunks x [128,35]  ([Wv|Wz] order)
    take("ae1", 64)           # [97,64]
    take("ae2", 32)           # [64,32]
    take("g1", 64)            # [33,64] stored at partitions 64..96
    take("g2", 5)             # [64,5]
    take("g2r1", 128)         # [64,128] replicated gate cols, experts 0..3
    take("g2r2", 29)          # [64,29]  replicated gate col, expert 4
    take("e1a", 5 * 128)      # [99,128] x5
    take("e1b", 5 * 128)      # [97,128] x5
    take("c2", 5 * 128)       # [1,128] x5 (unused now)
    take("e2", 5 * 128)       # [128,128] x5
    take("e3", 5 * 32)        # [128,32] x5 (padded to 32)
    take("msum", 29)          # [128,29] 0/1 block-sum matrix
    take("i29", 29)           # [29,29] identity
    assert c <= WCOLS, c
    return off


WOFF = _w_offsets()

# bpack columns
BC_ZV = 0      # rows 0..34:  [bv|bz]' adjusted
BC_ZE = 1      # rows 0..31:  ae_b2'
BC_G2 = 2      # rows 0..4:   gate_b2'
BC_NEG1 = 3    # all rows: -1.0
BC_G2R = 4     # rows 32e+k (e<4,k<29): gate_b2'_e   (replicated-logit bias)
BC_G2R4 = 5    # rows 0..28: gate_b2'_4
BC_B3 = 6      # rows 32e+k (e<4,k<29): b3'_e[k]
BC_B34 = 7     # rows 0..28: b3'_4
BC_C2 = 8      # cols 8..12:  expert-L2 bias c2_e (rows 0..127)
BC_C2P1 = 13   # cols 13..17: c2_e + 1
NBCOLS = 18

# (1+y/n)^n exp-approx constants, numerically tuned per pre-activation
# sigma to minimize rms elu error (see fit in dev notes).
EA16, EB16 = 0.0600, 0.9395          # VAE sites (sigma ~1.1)
EA16S, EB16S = 0.060625, 0.9390      # small-sigma sites (experts, AE, gate)
EA8, EB8 = 0.11625, 0.8820           # n=8 expert-L2 sites


# ------------------------------------------------------- custom DVE elu ops

_ELU_OPS = {}


def _register_elu_ops():
    """ELU16_ANT: in0 = y+1 -> max(min((in0*s0+s1)^16, 1), in0)
    ELU8B_ANT:   s = in0 + s0(per-part bias+1) -> max(min((s*s1+imm2)^8,1), s)
    """
    if _ELU_OPS:
        return _ELU_OPS
    import concourse.dve_ops as dve_ops
    from concourse.dve_spec import Spec, Src0, C0, C1, C2, One, maxx, minn, sq, lower
    from concourse.dve_ops import DveOp
    from concourse.dve_uop import DveOpSpec

    def make(name, body, ref):
        if name in dve_ops._SUB_OPCODE_FOR_NAME:
            for op in dve_ops.OPS:
                if op.name == name:
                    return op
        spec = Spec(body=body, reference=ref)
        row = max(dve_ops._SUB_OPCODE_FOR_NAME.values()) + 1
        assert row < 0x20
        shas = {}
        for ver in ("v3", "v4"):
            s = DveOpSpec(name=name, opcode=row, uops=lower(spec, ver=ver),
                          rd1_en=False)
            shas[ver] = s.sha(ver)
        op = DveOp(name, spec, subdim=False, uops_sha=shas)
        dve_ops.OPS.append(op)
        dve_ops.CUSTOM_DVE_SPECS[name] = spec
        dve_ops._SUB_OPCODE_FOR_NAME[name] = row
        return op

    b16 = maxx(minn(sq(sq(sq(sq(Src0 * C0 + C1)))), One), Src0)
    _ELU_OPS["elu16"] = make(
        "ELU16_ANT", b16,
        lambda in0, in1, s0, s1, imm2: np.maximum(
            np.minimum((in0 * s0 + s1) ** 16, 1.0), in0),
    )
    _s = Src0 + C0
    b8 = maxx(minn(sq(sq(sq(_s * C1 + C2))), One), _s)
    _ELU_OPS["elu8b"] = make(
        "ELU8B_ANT", b8,
        lambda in0, in1, s0, s1, imm2: np.maximum(
            np.minimum(((in0 + s0) * s1 + imm2) ** 8, 1.0), in0 + s0),
    )
    return _ELU_OPS


# ----------------------------------------------------------------- device IR

def build_program(n_rows=N_CORE, num_devices=N_CORES):
    """Build + compile the per-core Bass program. Returns nc."""
    import concourse.bass as bass
    import concourse.mybir as mybir
    from concourse import bacc
    from concourse.tile import TileContext

    ops = _register_elu_ops()
    ELU16, ELU8B = ops["elu16"], ops["elu8b"]

    fp16 = mybir.dt.float16
    fp32 = mybir.dt.float32
    AF = mybir.ActivationFunctionType
    OP = mybir.AluOpType

    n_tiles = n_rows // TILE
    assert n_rows % TILE == 0

    nc = bacc.Bacc("TRN2", target_bir_lowering=False, debug=False,
                   num_devices=num_devices)

    xT = nc.dram_tensor("xT", (XT_ROWS, n_rows), fp16, kind="ExternalInput").ap()
    wpack = nc.dram_tensor("wpack", (128, WCOLS), fp16, kind="ExternalInput").ap()
    bpack = nc.dram_tensor("bpack", (128, NBCOLS), fp32, kind="ExternalInput").ap()
    out_fm = nc.dram_tensor("out_fm", (29, n_rows), fp32, kind="ExternalOutput").ap()
    gl_fm = nc.dram_tensor("gl_fm", (5, n_rows), fp16, kind="ExternalOutput").ap()

    with TileContext(nc) as tc:
        with (
            tc.tile_pool(name="const", bufs=1) as constp,
            tc.tile_pool(name="xio", bufs=4) as xio,
            tc.tile_pool(name="inp", bufs=4) as inpp,
            tc.tile_pool(name="uh", bufs=3) as uhp,
            tc.tile_pool(name="small", bufs=8) as smallp,
            tc.tile_pool(name="u12", bufs=8) as u12p,
            tc.tile_pool(name="blend", bufs=6) as blendp,
            tc.tile_pool(name="pe", bufs=2, space="PSUM") as pep,
            tc.tile_pool(name="pmain", bufs=3, space="PSUM") as pmainp,
            tc.tile_pool(name="pacts", bufs=1, space="PSUM") as pactsp,
        ):
            # ---- persistent constants
            wsb = constp.tile([128, WCOLS], fp16, tag="wsb")
            nc.sync.dma_start(out=wsb, in_=wpack)
            bsb = constp.tile([128, NBCOLS], fp32, tag="bsb")
            nc.sync.dma_start(out=bsb, in_=bpack)

            xT_blk = xT[0:640].rearrange("(b p) n -> p b n", p=128)  # [128, 5, n]

            def w(name, k, m, idx=0, msz=None, prow=0):
                base = WOFF[name] + idx * m
                return wsb[prow:prow + k, base:base + (msz if msz is not None else m)]

            def bcol(col, m=128, r0=0):
                return bsb[r0:r0 + m, col:col + 1]

            def elu16(pool, tag, src, m, fd=TILE, a=EA16S, b=EB16S):
                """src[0:m, 0:fd] holds y+1 -> u = elu(y)+1 fp16 (one DVE op)."""
                u = pool.tile([128, 2 * TILE], fp16, tag=tag)
                nc.vector._custom_dve(ELU16, out=u[0:m, 0:fd], in0=src[0:m, 0:fd],
                                      s0=a, s1=b)
                return u

            zero_b = None  # memset-zero bias column? use bsb col of zeros
            # bpack has no all-zero column guaranteed... BC_ZV rows35+ are 0,
            # safer: make a zeros tile once
            zb = constp.tile([128, 1], fp32, tag="zb")
            nc.vector.memset(zb, 0.0)

            pending_blend = [None]
            for it in range(n_tiles):
                n0 = it * TILE
                # ---- loads
                xsb = xio.tile([128, 5, TILE], fp16, tag="xsb")
                nc.sync.dma_start(out=xsb, in_=xT_blk[:, 0:5, n0:n0 + TILE])
                inpA = inpp.tile([128, TILE], fp16, tag="inpA")
                nc.sync.dma_start(out=inpA, in_=xT[640:768, n0:n0 + TILE])
                inpB = inpp.tile([128, TILE], fp16, tag="inpB")
                nc.sync.dma_start(out=inpB[32:64], in_=xT[739:771, n0:n0 + TILE])
                nc.sync.dma_start(out=inpB[97:128], in_=xT[739:770, n0:n0 + TILE])
                nc.sync.dma_start(out=inpB[96:97], in_=xT[771:772, n0:n0 + TILE])

                # early expert-L1 A-chunks: only need inpA (o_t DMA)
                pe1a = pep.tile([128, 2 * TILE], fp32, tag="pe")
                pe1b = pep.tile([128, 2 * TILE], fp32, tag="pe")
                for pe1_, pair_ in ((pe1a, (0, 1)), (pe1b, (2, 3))):
                    for j_, e_ in enumerate(pair_):
                        nc.tensor.matmul(pe1_[:, j_ * TILE:(j_ + 1) * TILE],
                                         lhsT=w("e1a", 128, 128, e_),
                                         rhs=inpA, start=True, stop=False)

                if pending_blend[0] is not None:
                    pending_blend[0]()
                    pending_blend[0] = None

                # ---- VAE L1: two 128-halves, separate psum tiles
                u_h = uhp.tile([128, 2 * TILE], fp16, tag="uh")
                for half in (0, 1):
                    ph = pmainp.tile([128, TILE], fp32, tag="pmain")
                    for c in range(4):
                        nc.tensor.matmul(
                            ph,
                            lhsT=wsb[0:128, WOFF["w1"] + c * 256 + half * 128:
                                     WOFF["w1"] + c * 256 + half * 128 + 128],
                            rhs=xsb[0:128, c, :],
                            start=(c == 0), stop=(c == 3))
                    nc.vector._custom_dve(
                        ELU16, out=u_h[:, half * TILE:(half + 1) * TILE],
                        in0=ph[:, 0:TILE], s0=EA16, s1=EB16)

                # ---- VAE L2 -> [v_pred(3) | z_H(32)] into inpB[0:35]
                pz = pmainp.tile([128, TILE], fp32, tag="pmain")
                nc.tensor.matmul(pz[0:35], lhsT=w("wzv", 128, 35, 0),
                                 rhs=u_h[:, 0:TILE], start=True, stop=False)
                nc.tensor.matmul(pz[0:35], lhsT=w("wzv", 128, 35, 1),
                                 rhs=u_h[:, TILE:2 * TILE], start=False, stop=True)
                nc.scalar.activation(inpB[0:35], pz[0:35], AF.Identity,
                                     bias=bcol(BC_ZV, 35), scale=1.0)

                # ---- AE: ha -> z_E(32) into inpB[64:96]  (ACT-heavy elu path)
                pa = pmainp.tile([128, TILE], fp32, tag="pmain")
                nc.tensor.matmul(pa[0:64], lhsT=w("ae1", 128, 64), rhs=xsb[0:128, 4, :],
                                 start=True, stop=True)
                u_a = elu16(smallp, "ua", pa, 64)
                pzE = pmainp.tile([128, TILE], fp32, tag="pmain")
                nc.tensor.matmul(pzE[0:32], lhsT=w("ae2", 64, 32), rhs=u_a[0:64, 0:TILE],
                                 start=True, stop=True)
                nc.scalar.activation(inpB[64:96], pzE[0:32], AF.Identity,
                                     bias=bcol(BC_ZE, 32), scale=1.0)

                # ---- experts: L1 B-chunks for 0..3
                for pe1_, pair_ in ((pe1a, (0, 1)), (pe1b, (2, 3))):
                    for j_, e_ in enumerate(pair_):
                        nc.tensor.matmul(pe1_[:, j_ * TILE:(j_ + 1) * TILE],
                                         lhsT=w("e1b", 128, 128, e_),
                                         rhs=inpB, start=False, stop=True)

                # L1 elu + L2 for pair A then pair B
                u1a = elu16(u12p, "u12", pe1a, 128, 2 * TILE)
                pe2a = pep.tile([128, 2 * TILE], fp32, tag="pe")
                for j, e in enumerate((0, 1)):
                    sl = slice(j * TILE, (j + 1) * TILE)
                    nc.tensor.matmul(pe2a[:, sl], lhsT=w("e2", 128, 128, e),
                                     rhs=u1a[:, sl], start=True, stop=True)
                u1b = elu16(u12p, "u12", pe1b, 128, 2 * TILE)
                pe2b = pep.tile([128, 2 * TILE], fp32, tag="pe")
                for j, e in enumerate((2, 3)):
                    sl = slice(j * TILE, (j + 1) * TILE)
                    nc.tensor.matmul(pe2b[:, sl], lhsT=w("e2", 128, 128, e),
                                     rhs=u1b[:, sl], start=True, stop=True)

                # ---- gate chain (overlaps expert L2/elu work)
                pg = pmainp.tile([128, TILE], fp32, tag="pmain")
                nc.tensor.matmul(pg[0:64], lhsT=w("g1", 33, 64, prow=64),
                                 rhs=inpB[64:97], start=True, stop=True)
                u_g = elu16(smallp, "ug", pg, 64)
                pgl = pmainp.tile([128, TILE], fp32, tag="pmain")
                nc.tensor.matmul(pgl[0:5], lhsT=w("g2", 64, 5), rhs=u_g[0:64, 0:TILE],
                                 start=True, stop=True)
                glq = smallp.tile([5, TILE], fp16, tag="glq")
                nc.scalar.activation(glq, pgl[0:5], AF.Identity,
                                     bias=bcol(BC_G2, 5), scale=1.0)
                nc.sync.dma_start(out=gl_fm[:, n0:n0 + TILE], in_=glq)
                pglR = pmainp.tile([128, TILE], fp32, tag="pmain")
                nc.tensor.matmul(pglR, lhsT=w("g2r1", 64, 128), rhs=u_g[0:64, 0:TILE],
                                 start=True, stop=True)
                eg = blendp.tile([128, TILE], fp16, tag="eg")
                nc.scalar.activation(eg, pglR, AF.Exp,
                                     bias=bcol(BC_G2R), scale=1.0)
                pglR4 = pmainp.tile([128, TILE], fp32, tag="pmain")
                nc.tensor.matmul(pglR4[0:29], lhsT=w("g2r2", 64, 29),
                                 rhs=u_g[0:64, 0:TILE], start=True, stop=True)
                eg4 = blendp.tile([29, TILE], fp16, tag="eg4")
                nc.scalar.activation(eg4, pglR4[0:29], AF.Exp,
                                     bias=bcol(BC_G2R4, 29), scale=1.0)

                # ---- expert L2 elu (per-expert bias via ELU8B) + L3 col-tiled
                pacts0 = pactsp.tile([128, TILE], fp32, tag="pacts")
                u2a = u12p.tile([128, 2 * TILE], fp16, tag="u12")
                u2b = u12p.tile([128, 2 * TILE], fp16, tag="u12")
                for u2_, pe2_, pair_ in ((u2a, pe2a, (0, 1)), (u2b, pe2b, (2, 3))):
                    for j, e in enumerate(pair_):
                        sl = slice(j * TILE, (j + 1) * TILE)
                        nc.vector._custom_dve(
                            ELU8B, out=u2_[:, sl], in0=pe2_[:, sl],
                            s0=bcol(BC_C2P1 + e), s1=EA8, imm2=EB8)
                # all four L3 matmuls back-to-back -> col-tile concurrency
                for e in range(4):
                    u2_ = u2a if e < 2 else u2b
                    sl = slice((e % 2) * TILE, (e % 2 + 1) * TILE)
                    nc.tensor.matmul(pacts0[32 * e:32 * e + 32],
                                     lhsT=w("e3", 128, 32, e), rhs=u2_[:, sl],
                                     start=True, stop=True,
                                     tile_position=(0, 32 * e))

                # ---- expert 4 single lane
                pe14 = pmainp.tile([128, TILE], fp32, tag="pmain")
                nc.tensor.matmul(pe14, lhsT=w("e1a", 128, 128, 4),
                                 rhs=inpA, start=True, stop=False)
                nc.tensor.matmul(pe14, lhsT=w("e1b", 128, 128, 4),
                                 rhs=inpB, start=False, stop=True)
                u14 = elu16(u12p, "u14", pe14, 128)
                pe24 = pmainp.tile([128, TILE], fp32, tag="pmain")
                nc.tensor.matmul(pe24, lhsT=w("e2", 128, 128, 4),
                                 rhs=u14[:, 0:TILE], start=True, stop=True)
                u24 = u12p.tile([128, TILE], fp16, tag="u24")
                nc.vector._custom_dve(ELU8B, out=u24[:, 0:TILE], in0=pe24[:, 0:TILE],
                                      s0=bcol(BC_C2P1 + 4), s1=EA8, imm2=EB8)
                pacts1 = pmainp.tile([128, TILE], fp32, tag="pmain")
                nc.tensor.matmul(pacts1[0:29], lhsT=w("e3", 128, 32, 4, msz=29),
                                 rhs=u24[:, 0:TILE], start=True, stop=True)

                def make_blend(bn0, pacts0, pacts1, eg, eg4):
                    def emit_blend():
                        # acts+bias on ACT, x gate-weight on DVE (2x fp16)
                        a_all = blendp.tile([128, TILE], fp16, tag="a_all")
                        nc.scalar.activation(a_all, pacts0, AF.Identity,
                                             bias=bcol(BC_B3), scale=1.0)
                        s_all = blendp.tile([128, TILE], fp16, tag="s_all")
                        nc.vector.tensor_tensor(out=s_all, in0=a_all, in1=eg,
                                                op=OP.mult)
                        a4 = blendp.tile([29, TILE], fp16, tag="a4")
                        nc.scalar.activation(a4, pacts1[0:29], AF.Identity,
                                             bias=bcol(BC_B34, 29), scale=1.0)
                        se4 = blendp.tile([29, TILE], fp16, tag="se4")
                        nc.vector.tensor_tensor(out=se4, in0=a4, in1=eg4,
                                                op=OP.mult)
                        pbl = pmainp.tile([128, TILE], fp32, tag="pmain")
                        nc.tensor.matmul(pbl[0:29], lhsT=w("msum", 128, 29), rhs=s_all,
                                         start=True, stop=False)
                        nc.tensor.matmul(pbl[0:29], lhsT=w("i29", 29, 29), rhs=se4,
                                         start=False, stop=True)
                        acc = blendp.tile([29, TILE], fp32, tag="acc")
                        nc.scalar.activation(acc, pbl[0:29], AF.Identity,
                                             bias=zb[0:29], scale=1.0)
                        nc.sync.dma_start(out=out_fm[:, bn0:bn0 + TILE], in_=acc)
                    return emit_blend
                pending_blend[0] = make_blend(n0, pacts0, pacts1, eg, eg4)

            if pending_blend[0] is not None:
                pending_blend[0]()
    nc.compile()
    return nc


# ----------------------------------------------------------------- host prep

def prep_inputs(x, vae_W1, vae_b1, vae_Wz, vae_bz, vae_Wv, vae_bv,
                ae_W1, ae_b1, ae_W2, ae_b2,
                gate_W1, gate_b1, gate_W2, gate_b2,
                eW1, eb1, eW2, eb2, eW3, eb3, n_rows=N_CORE, n_cores=N_CORES):
    """Returns in_maps (list of per-core dicts)."""
    x = np.asarray(x, np.float32)
    n_total = n_rows * n_cores
    assert x.shape[0] >= n_total

    xT = np.zeros((XT_ROWS, n_total), np.float16)
    xv = x[:n_total, VAE_COLS].T.astype(np.float16)  # [480, n]
    for c in range(4):
        xT[128 * c:128 * c + 120] = xv[120 * c:120 * c + 120]
    xT[504] = 1.0
    xT[512:608] = x[:n_total, ELEV_COLS].T.astype(np.float16)
    xT[608] = 1.0
    xT[640:739] = x[:n_total, OT_COLS].T.astype(np.float16)
    xT[771] = 1.0

    wpack = np.zeros((128, WCOLS), np.float32)
    bpack = np.zeros((128, NBCOLS), np.float32)
    bpack[:, BC_NEG1] = -1.0

    def put(name, idx, arr, msz=None, prow=0):
        k, m = arr.shape
        base = WOFF[name] + idx * (msz if msz is not None else m)
        wpack[prow:prow + k, base:base + m] = arr

    W1 = np.asarray(vae_W1, np.float32)  # [480, 256] rows already in vae_hist order
    for c in range(4):
        chunk = W1[120 * c:120 * c + 120]
        if c == 3:
            chunk = np.vstack([chunk, (np.asarray(vae_b1) + 1.0)[None]])
        put("w1", c, chunk, msz=256)
    # [Wv | Wz] order so the evac lands [v_pred(3) | z_H(32)] at inpB[0:35]
    Wzv = np.concatenate([vae_Wv, vae_Wz], axis=1).astype(np.float32)  # [256,35]
    put("wzv", 0, Wzv[0:128], msz=35)
    put("wzv", 1, Wzv[128:256], msz=35)
    bpack[0:35, BC_ZV] = np.concatenate([vae_bv, vae_bz]) - Wzv.sum(0)

    put("ae1", 0, np.vstack([ae_W1, (np.asarray(ae_b1) + 1.0)[None]]))
    put("ae2", 0, np.asarray(ae_W2, np.float32))
    bpack[0:32, BC_ZE] = np.asarray(ae_b2) - np.asarray(ae_W2).sum(0)

    put("g1", 0, np.vstack([gate_W1, (np.asarray(gate_b1) + 1.0)[None]]), prow=64)
    G2 = np.asarray(gate_W2, np.float32)  # [64,5]
    put("g2", 0, G2)
    bg2 = np.asarray(gate_b2) - G2.sum(0)  # [5]
    bpack[0:5, BC_G2] = bg2
    g2r1 = np.zeros((64, 128), np.float32)
    for e in range(4):
        g2r1[:, 32 * e:32 * e + 29] = G2[:, e:e + 1]
        bpack[32 * e:32 * e + 29, BC_G2R] = bg2[e]
    put("g2r1", 0, g2r1)
    g2r2 = np.repeat(G2[:, 4:5], 29, axis=1)
    put("g2r2", 0, g2r2)
    bpack[0:29, BC_G2R4] = bg2[4]

    for e in range(5):
        W1e = np.asarray(eW1[e], np.float32)  # [166,128]
        put("e1a", e, W1e[0:99], msz=128)
        e1b = np.zeros((97, 128), np.float32)
        e1b[0:35] = W1e[99:134]      # v_pred(3) + z_H(32) weight rows
        e1b[64:96] = W1e[134:166]    # z_E rows
        e1b[96] = np.asarray(eb1[e]) + 1.0
        put("e1b", e, e1b, msz=128)
        W2e = np.asarray(eW2[e], np.float32)
        c2 = np.asarray(eb2[e]) - W2e.sum(0)
        bpack[0:128, BC_C2 + e] = c2
        bpack[0:128, BC_C2P1 + e] = c2 + 1.0
        put("e2", e, W2e, msz=128)
        W3e = np.asarray(eW3[e], np.float32)
        W3p = np.zeros((128, 32), np.float32)
        W3p[:, 0:29] = W3e
        put("e3", e, W3p, msz=32)
        b3e = np.asarray(eb3[e]) - W3e.sum(0)  # [29]
        if e < 4:
            bpack[32 * e:32 * e + 29, BC_B3] = b3e
        else:
            bpack[0:29, BC_B34] = b3e
    msum = np.zeros((128, 29), np.float32)
    for e in range(4):
        msum[32 * e:32 * e + 29] = np.eye(29)
    put("msum", 0, msum)
    put("i29", 0, np.eye(29, dtype=np.float32))

    wpack16 = wpack.astype(np.float16)
    in_maps = []
    for c in range(n_cores):
        in_maps.append({
            "xT": np.ascontiguousarray(xT[:, c * n_rows:(c + 1) * n_rows]),
            "wpack": wpack16,
            "bpack": bpack,
        })
    return in_maps


# ----------------------------------------------------------------- entry

_NC_CACHE = {}


def _get_program(n_rows=N_CORE, num_devices=N_CORES):
    key = (n_rows, num_devices)
    if key not in _NC_CACHE:
        _NC_CACHE[key] = build_program(n_rows, num_devices)
    return _NC_CACHE[key]


def kernel(**inputs):
    from concourse.bass_utils import run_bass_kernel_spmd

    nc = _get_program()
    in_maps = prep_inputs(**inputs)
    res = run_bass_kernel_spmd(nc, in_maps, core_ids=list(range(N_CORES)))
    out = np.empty((N_FULL, 29), np.float32)
    for c in range(N_CORES):
        num = res.results[c]["out_fm"]            # [29, n] unnormalized
        gl = res.results[c]["gl_fm"].astype(np.float32)  # [5, n]
        den = np.exp(gl).sum(axis=0)              # softmax denominator
        out[c * N_CORE:(c + 1) * N_CORE] = (num / den[None, :]).T
    return out


# revision 12
# speedup vs baseline: 1.7505x; 1.1886x over previous
"""Trainium2 Bass kernel for nn_MoEAugmentedActor (moe_routing).

Strategy (pure data parallel, 8 cores, batch-sharded):
  - Host prepares a feature-major fp16 view xT of the needed x columns
    (579 of 975), with ones-rows baked in so L1 biases ride the matmul.
  - On-chip everything is feature-major: [features(part), batch(free)],
    batch tiled at 512 columns.
  - ELU(y)+1 is computed by a single fused custom DVE op:
      u = max(min((a*(y+1) + b)^16, 1), y+1)   with a=1/16, b=15/16,
    i.e. (1+y/16)^16 ~= e^y (4 squarings).  One DVE pass per site,
    no ScalarE exp, no second select pass.  A variant (ELU8B) folds a
    per-partition bias add in and uses 3 squarings (n=8) for the
    expert-L2 sites whose bias cannot ride the matmul.
  - Three small sites (AE, gate hidden, expert-4 L1) instead use
    ScalarE exp + evac and a 2x-mode fp16 stock scalar_tensor_tensor,
    to balance load between ScalarE and VectorE.
  - Softmax over the 5 gate logits is NOT normalized on device: the
    kernel exports the fp16 logits and blends with unnormalized
    exp-weights; the host divides by the softmax denominator.
  - Device writes out feature-major [29, n] numerators + [5, n] logits;
    host normalizes and transposes back.
"""

import os
import sys

for _p in ("/opt/trn_rl_repo", "/root/.axon_site/_ro/trn_rl_repo"):
    if os.path.isdir(_p) and _p not in sys.path:
        sys.path.insert(0, _p)

import numpy as np

# ----------------------------------------------------------------- constants
N_FULL = 131072
N_CORES = 8
N_CORE = N_FULL // N_CORES  # 16384
TILE = 512  # batch columns per tile

OBS_TERM_DIMS = (3, 3, 3, 3, 29, 29, 29, 96)
HISTORY_LEN = 5
_OFFS = [0]
for _d in OBS_TERM_DIMS[:-1]:
    _OFFS.append(_OFFS[-1] + _d * HISTORY_LEN)

# vae_hist column order: frame i in 0..4, terms 1..6, dims within term
VAE_COLS = [
    _OFFS[t] + i * OBS_TERM_DIMS[t] + j
    for i in range(HISTORY_LEN)
    for t in range(1, 7)
    for j in range(OBS_TERM_DIMS[t])
]  # 480
OT_COLS = [
    _OFFS[t] + 4 * OBS_TERM_DIMS[t] + j for t in range(7) for j in range(OBS_TERM_DIMS[t])
]  # 99
ELEV_COLS = list(range(_OFFS[7] + 4 * 96, _OFFS[7] + 5 * 96))  # 96

XT_ROWS = 784  # 6 blocks of 128 + zeros/ones tail
WCOLS = 4224
X8_ROWS = 100   # o_t(99) + ones row, fp8e4
XV8_SLOTS = 4   # vae_hist in 4 k-tiles of 128 rows (481 used), fp8e4
W8COLS = 5 * 256 + 4 * 256  # expert-L1 DR weights + VAE-L1 DR weights


def _w_offsets():
    off = {}
    c = 0

    def take(name, n):
        nonlocal c
        off[name] = c
        c += n

    take("w1", 4 * 256)       # 4 chunks x [K,256]
    take("wzv", 2 * 35)       # 2 chunks x [128,35]  ([Wv|Wz] order)
    take("ae1", 64)           # [97,64]
    take("ae2", 32)           # [64,32]
    take("g1", 64)            # [33,64] stored at partitions 64..96
    take("g2", 5)             # [64,5]
    take("g2r1", 128)         # [64,128] replicated gate cols, experts 0..3
    take("g2r2", 29)          # [64,29]  replicated gate col, expert 4
    take("e1a", 5 * 128)      # [99,128] x5
    take("e1b", 5 * 128)      # [97,128] x5
    take("c2", 5 * 128)       # [1,128] x5 (unused now)
    take("e2", 5 * 128)       # [128,128] x5
    take("e3", 5 * 32)        # [128,32] x5 (padded to 32)
    take("msum", 29)          # [128,29] 0/1 block-sum matrix
    take("i29", 29)           # [29,29] identity
    assert c <= WCOLS, c
    return off


WOFF = _w_offsets()

# bpack columns
BC_ZV = 0      # rows 0..34:  [bv|bz]' adjusted
BC_ZE = 1      # rows 0..31:  ae_b2'
BC_G2 = 2      # rows 0..4:   gate_b2'
BC_NEG1 = 3    # all rows: -1.0
BC_G2R = 4     # rows 32e+k (e<4,k<29): gate_b2'_e   (replicated-logit bias)
BC_G2R4 = 5    # rows 0..28: gate_b2'_4
BC_B3 = 6      # rows 32e+k (e<4,k<29): b3'_e[k]
BC_B34 = 7     # rows 0..28: b3'_4
BC_C2 = 8      # cols 8..12:  expert-L2 bias c2_e (rows 0..127)
BC_C2P1 = 13   # cols 13..17: c2_e + 1
NBCOLS = 18

# (1+y/n)^n exp-approx constants, numerically tuned per pre-activation
# sigma to minimize rms elu error (see fit in dev notes).
EA16, EB16 = 0.0600, 0.9395          # VAE sites (sigma ~1.1)
EA16S, EB16S = 0.060625, 0.9390      # small-sigma sites (experts, AE, gate)
EA8, EB8 = 0.11625, 0.8820           # n=8 expert-L2 sites


# ------------------------------------------------------- custom DVE elu ops

_ELU_OPS = {}


def _register_elu_ops():
    """ELU16_ANT: in0 = y+1 -> max(min((in0*s0+s1)^16, 1), in0)
    ELU8B_ANT:   s = in0 + s0(per-part bias+1) -> max(min((s*s1+imm2)^8,1), s)
    """
    if _ELU_OPS:
        return _ELU_OPS
    import concourse.dve_ops as dve_ops
    from concourse.dve_spec import Spec, Src0, C0, C1, C2, One, maxx, minn, sq, lower
    from concourse.dve_ops import DveOp
    from concourse.dve_uop import DveOpSpec

    def make(name, body, ref):
        if name in dve_ops._SUB_OPCODE_FOR_NAME:
            for op in dve_ops.OPS:
                if op.name == name:
                    return op
        spec = Spec(body=body, reference=ref)
        row = max(dve_ops._SUB_OPCODE_FOR_NAME.values()) + 1
        assert row < 0x20
        shas = {}
        for ver in ("v3", "v4"):
            s = DveOpSpec(name=name, opcode=row, uops=lower(spec, ver=ver),
                          rd1_en=False)
            shas[ver] = s.sha(ver)
        op = DveOp(name, spec, subdim=False, uops_sha=shas)
        dve_ops.OPS.append(op)
        dve_ops.CUSTOM_DVE_SPECS[name] = spec
        dve_ops._SUB_OPCODE_FOR_NAME[name] = row
        return op

    b16 = maxx(minn(sq(sq(sq(sq(Src0 * C0 + C1)))), One), Src0)
    _ELU_OPS["elu16"] = make(
        "ELU16_ANT", b16,
        lambda in0, in1, s0, s1, imm2: np.maximum(
            np.minimum((in0 * s0 + s1) ** 16, 1.0), in0),
    )
    _s = Src0 + C0
    b8 = maxx(minn(sq(sq(sq(_s * C1 + C2))), One), _s)
    _ELU_OPS["elu8b"] = make(
        "ELU8B_ANT", b8,
        lambda in0, in1, s0, s1, imm2: np.maximum(
            np.minimum(((in0 + s0) * s1 + imm2) ** 8, 1.0), in0 + s0),
    )
    return _ELU_OPS


# ----------------------------------------------------------------- device IR

def build_program(n_rows=N_CORE, num_devices=N_CORES):
    """Build + compile the per-core Bass program. Returns nc."""
    import concourse.bass as bass
    import concourse.mybir as mybir
    from concourse import bacc
    from concourse.tile import TileContext

    ops = _register_elu_ops()
    ELU16, ELU8B = ops["elu16"], ops["elu8b"]

    fp16 = mybir.dt.float16
    fp32 = mybir.dt.float32
    AF = mybir.ActivationFunctionType
    OP = mybir.AluOpType

    n_tiles = n_rows // TILE
    assert n_rows % TILE == 0

    nc = bacc.Bacc("TRN2", target_bir_lowering=False, debug=False,
                   num_devices=num_devices)

    xT = nc.dram_tensor("xT", (XT_ROWS, n_rows), fp16, kind="ExternalInput").ap()
    wpack = nc.dram_tensor("wpack", (128, WCOLS), fp16, kind="ExternalInput").ap()
    bpack = nc.dram_tensor("bpack", (128, NBCOLS), fp32, kind="ExternalInput").ap()
    out_fm = nc.dram_tensor("out_fm", (29, n_rows), fp32, kind="ExternalOutput").ap()
    gl_fm = nc.dram_tensor("gl_fm", (5, n_rows), fp16, kind="ExternalOutput").ap()

    with TileContext(nc) as tc:
        with (
            tc.tile_pool(name="const", bufs=1) as constp,
            tc.tile_pool(name="xio", bufs=4) as xio,
            tc.tile_pool(name="inp", bufs=4) as inpp,
            tc.tile_pool(name="uh", bufs=3) as uhp,
            tc.tile_pool(name="small", bufs=8) as smallp,
            tc.tile_pool(name="u12", bufs=8) as u12p,
            tc.tile_pool(name="blend", bufs=6) as blendp,
            tc.tile_pool(name="pe", bufs=2, space="PSUM") as pep,
            tc.tile_pool(name="pmain", bufs=3, space="PSUM") as pmainp,
            tc.tile_pool(name="pacts", bufs=1, space="PSUM") as pactsp,
        ):
            # ---- persistent constants
            wsb = constp.tile([128, WCOLS], fp16, tag="wsb")
            nc.sync.dma_start(out=wsb, in_=wpack)
            bsb = constp.tile([128, NBCOLS], fp32, tag="bsb")
            nc.sync.dma_start(out=bsb, in_=bpack)

            xT_blk = xT[0:640].rearrange("(b p) n -> p b n", p=128)  # [128, 5, n]

            def w(name, k, m, idx=0, msz=None, prow=0):
                base = WOFF[name] + idx * m
                return wsb[prow:prow + k, base:base + (msz if msz is not None else m)]

            def bcol(col, m=128, r0=0):
                return bsb[r0:r0 + m, col:col + 1]

            def elu16(pool, tag, src, m, fd=TILE, a=EA16S, b=EB16S):
                """src[0:m, 0:fd] holds y+1 -> u = elu(y)+1 fp16 (one DVE op)."""
                u = pool.tile([128, 2 * TILE], fp16, tag=tag)
                nc.vector._custom_dve(ELU16, out=u[0:m, 0:fd], in0=src[0:m, 0:fd],
                                      s0=a, s1=b)
                return u

            zero_b = None  # memset-zero bias column? use bsb col of zeros
            # bpack has no all-zero column guaranteed... BC_ZV rows35+ are 0,
            # safer: make a zeros tile once
            zb = constp.tile([128, 1], fp32, tag="zb")
            nc.vector.memset(zb, 0.0)

            def frontend(it):
                n0 = it * TILE
                xsb = xio.tile([128, 5, TILE], fp16, tag="xsb")
                nc.sync.dma_start(out=xsb, in_=xT_blk[:, 0:5, n0:n0 + TILE])
                inpA = inpp.tile([128, TILE], fp16, tag="inpA")
                nc.sync.dma_start(out=inpA, in_=xT[640:768, n0:n0 + TILE])
                inpB = inpp.tile([128, TILE], fp16, tag="inpB")
                nc.sync.dma_start(out=inpB[32:64], in_=xT[739:771, n0:n0 + TILE])
                nc.sync.dma_start(out=inpB[97:128], in_=xT[739:770, n0:n0 + TILE])
                nc.sync.dma_start(out=inpB[96:97], in_=xT[771:772, n0:n0 + TILE])

                # ---- VAE L1: two 128-halves
                u_h = uhp.tile([128, 2 * TILE], fp16, tag="uh")
                for half in (0, 1):
                    ph = pmainp.tile([128, TILE], fp32, tag="pmain")
                    for c in range(4):
                        nc.tensor.matmul(
                            ph,
                            lhsT=wsb[0:128, WOFF["w1"] + c * 256 + half * 128:
                                     WOFF["w1"] + c * 256 + half * 128 + 128],
                            rhs=xsb[0:128, c, :],
                            start=(c == 0), stop=(c == 3))
                    nc.vector._custom_dve(
                        ELU16, out=u_h[:, half * TILE:(half + 1) * TILE],
                        in0=ph[:, 0:TILE], s0=EA16, s1=EB16)

                # ---- VAE L2 -> [v_pred(3) | z_H(32)] into inpB[0:35]
                pz = pmainp.tile([128, TILE], fp32, tag="pmain")
                nc.tensor.matmul(pz[0:35], lhsT=w("wzv", 128, 35, 0),
                                 rhs=u_h[:, 0:TILE], start=True, stop=False)
                nc.tensor.matmul(pz[0:35], lhsT=w("wzv", 128, 35, 1),
                                 rhs=u_h[:, TILE:2 * TILE], start=False, stop=True)
                nc.scalar.activation(inpB[0:35], pz[0:35], AF.Identity,
                                     bias=bcol(BC_ZV, 35), scale=1.0)

                # ---- AE: ha -> z_E(32) into inpB[64:96]
                pa = pmainp.tile([128, TILE], fp32, tag="pmain")
                nc.tensor.matmul(pa[0:64], lhsT=w("ae1", 128, 64), rhs=xsb[0:128, 4, :],
                                 start=True, stop=True)
                u_a = elu16(smallp, "ua", pa, 64)
                pzE = pmainp.tile([128, TILE], fp32, tag="pmain")
                nc.tensor.matmul(pzE[0:32], lhsT=w("ae2", 64, 32), rhs=u_a[0:64, 0:TILE],
                                 start=True, stop=True)
                nc.scalar.activation(inpB[64:96], pzE[0:32], AF.Identity,
                                     bias=bcol(BC_ZE, 32), scale=1.0)

                # ---- gate chain
                pg = pmainp.tile([128, TILE], fp32, tag="pmain")
                nc.tensor.matmul(pg[0:64], lhsT=w("g1", 33, 64, prow=64),
                                 rhs=inpB[64:97], start=True, stop=True)
                u_g = elu16(smallp, "ug", pg, 64)
                pgl = pmainp.tile([128, TILE], fp32, tag="pmain")
                nc.tensor.matmul(pgl[0:5], lhsT=w("g2", 64, 5), rhs=u_g[0:64, 0:TILE],
                                 start=True, stop=True)
                glq = smallp.tile([5, TILE], fp16, tag="glq")
                nc.scalar.activation(glq, pgl[0:5], AF.Identity,
                                     bias=bcol(BC_G2, 5), scale=1.0)
                nc.sync.dma_start(out=gl_fm[:, n0:n0 + TILE], in_=glq)
                pglR = pmainp.tile([128, TILE], fp32, tag="pmain")
                nc.tensor.matmul(pglR, lhsT=w("g2r1", 64, 128), rhs=u_g[0:64, 0:TILE],
                                 start=True, stop=True)
                eg = blendp.tile([128, TILE], fp16, tag="eg")
                nc.scalar.activation(eg, pglR, AF.Exp,
                                     bias=bcol(BC_G2R), scale=1.0)
                pglR4 = pmainp.tile([128, TILE], fp32, tag="pmain")
                nc.tensor.matmul(pglR4[0:29], lhsT=w("g2r2", 64, 29),
                                 rhs=u_g[0:64, 0:TILE], start=True, stop=True)
                eg4 = blendp.tile([29, TILE], fp16, tag="eg4")
                nc.scalar.activation(eg4, pglR4[0:29], AF.Exp,
                                     bias=bcol(BC_G2R4, 29), scale=1.0)
                return dict(n0=n0, inpA=inpA, inpB=inpB, eg=eg, eg4=eg4)

            def backend(st):
                n0, inpA, inpB = st["n0"], st["inpA"], st["inpB"]
                eg, eg4 = st["eg"], st["eg4"]
                pe1a = pep.tile([128, 2 * TILE], fp32, tag="pe")
                pe1b = pep.tile([128, 2 * TILE], fp32, tag="pe")
                for pe1_, pair_ in ((pe1a, (0, 1)), (pe1b, (2, 3))):
                    for j_, e_ in enumerate(pair_):
                        sl = slice(j_ * TILE, (j_ + 1) * TILE)
                        nc.tensor.matmul(pe1_[:, sl], lhsT=w("e1a", 128, 128, e_),
                                         rhs=inpA, start=True, stop=False)
                        nc.tensor.matmul(pe1_[:, sl], lhsT=w("e1b", 128, 128, e_),
                                         rhs=inpB, start=False, stop=True)

                u1a = elu16(u12p, "u12", pe1a, 128, 2 * TILE)
                pe2a = pep.tile([128, 2 * TILE], fp32, tag="pe")
                for j, e in enumerate((0, 1)):
                    sl = slice(j * TILE, (j + 1) * TILE)
                    nc.tensor.matmul(pe2a[:, sl], lhsT=w("e2", 128, 128, e),
                                     rhs=u1a[:, sl], start=True, stop=True)
                u1b = elu16(u12p, "u12", pe1b, 128, 2 * TILE)
                pe2b = pep.tile([128, 2 * TILE], fp32, tag="pe")
                for j, e in enumerate((2, 3)):
                    sl = slice(j * TILE, (j + 1) * TILE)
                    nc.tensor.matmul(pe2b[:, sl], lhsT=w("e2", 128, 128, e),
                                     rhs=u1b[:, sl], start=True, stop=True)

                pacts0 = pactsp.tile([128, TILE], fp32, tag="pacts")
                u2a = u12p.tile([128, 2 * TILE], fp16, tag="u12")
                u2b = u12p.tile([128, 2 * TILE], fp16, tag="u12")
                for u2_, pe2_, pair_ in ((u2a, pe2a, (0, 1)), (u2b, pe2b, (2, 3))):
                    for j, e in enumerate(pair_):
                        sl = slice(j * TILE, (j + 1) * TILE)
                        nc.vector._custom_dve(
                            ELU8B, out=u2_[:, sl], in0=pe2_[:, sl],
                            s0=bcol(BC_C2P1 + e), s1=EA8, imm2=EB8)
                for e in range(4):
                    u2_ = u2a if e < 2 else u2b
                    sl = slice((e % 2) * TILE, (e % 2 + 1) * TILE)
                    nc.tensor.matmul(pacts0[32 * e:32 * e + 32],
                                     lhsT=w("e3", 128, 32, e), rhs=u2_[:, sl],
                                     start=True, stop=True,
                                     tile_position=(0, 32 * e))

                # ---- expert 4 single lane
                pe14 = pmainp.tile([128, TILE], fp32, tag="pmain")
                nc.tensor.matmul(pe14, lhsT=w("e1a", 128, 128, 4),
                                 rhs=inpA, start=True, stop=False)
                nc.tensor.matmul(pe14, lhsT=w("e1b", 128, 128, 4),
                                 rhs=inpB, start=False, stop=True)
                u14 = elu16(u12p, "u14", pe14, 128)
                pe24 = pmainp.tile([128, TILE], fp32, tag="pmain")
                nc.tensor.matmul(pe24, lhsT=w("e2", 128, 128, 4),
                                 rhs=u14[:, 0:TILE], start=True, stop=True)
                u24 = u12p.tile([128, TILE], fp16, tag="u24")
                nc.vector._custom_dve(ELU8B, out=u24[:, 0:TILE], in0=pe24[:, 0:TILE],
                                      s0=bcol(BC_C2P1 + 4), s1=EA8, imm2=EB8)
                pacts1 = pmainp.tile([128, TILE], fp32, tag="pmain")
                nc.tensor.matmul(pacts1[0:29], lhsT=w("e3", 128, 32, 4, msz=29),
                                 rhs=u24[:, 0:TILE], start=True, stop=True)

                # ---- blend
                a_all = blendp.tile([128, TILE], fp16, tag="a_all")
                nc.scalar.activation(a_all, pacts0, AF.Identity,
                                     bias=bcol(BC_B3), scale=1.0)
                s_all = blendp.tile([128, TILE], fp16, tag="s_all")
                nc.vector.tensor_tensor(out=s_all, in0=a_all, in1=eg, op=OP.mult)
                a4 = blendp.tile([29, TILE], fp16, tag="a4")
                nc.scalar.activation(a4, pacts1[0:29], AF.Identity,
                                     bias=bcol(BC_B34, 29), scale=1.0)
                se4 = blendp.tile([29, TILE], fp16, tag="se4")
                nc.vector.tensor_tensor(out=se4, in0=a4, in1=eg4, op=OP.mult)
                pbl = pmainp.tile([128, TILE], fp32, tag="pmain")
                nc.tensor.matmul(pbl[0:29], lhsT=w("msum", 128, 29), rhs=s_all,
                                 start=True, stop=False)
                nc.tensor.matmul(pbl[0:29], lhsT=w("i29", 29, 29), rhs=se4,
                                 start=False, stop=True)
                acc = blendp.tile([29, TILE], fp32, tag="acc")
                nc.scalar.activation(acc, pbl[0:29], AF.Identity,
                                     bias=zb[0:29], scale=1.0)
                nc.sync.dma_start(out=out_fm[:, n0:n0 + TILE], in_=acc)

            prev = None
            for it in range(n_tiles):
                st = frontend(it)
                if prev is not None:
                    backend(prev)
                prev = st
            backend(prev)
    nc.compile()
    return nc


# ----------------------------------------------------------------- host prep

def prep_inputs(x, vae_W1, vae_b1, vae_Wz, vae_bz, vae_Wv, vae_bv,
                ae_W1, ae_b1, ae_W2, ae_b2,
                gate_W1, gate_b1, gate_W2, gate_b2,
                eW1, eb1, eW2, eb2, eW3, eb3, n_rows=N_CORE, n_cores=N_CORES):
    """Returns in_maps (list of per-core dicts)."""
    x = np.asarray(x, np.float32)
    n_total = n_rows * n_cores
    assert x.shape[0] >= n_total

    xT = np.zeros((XT_ROWS, n_total), np.float16)
    xv = x[:n_total, VAE_COLS].T.astype(np.float16)  # [480, n]
    for c in range(4):
        xT[128 * c:128 * c + 120] = xv[120 * c:120 * c + 120]
    xT[504] = 1.0
    xT[512:608] = x[:n_total, ELEV_COLS].T.astype(np.float16)
    xT[608] = 1.0
    xT[640:739] = x[:n_total, OT_COLS].T.astype(np.float16)
    xT[771] = 1.0

    wpack = np.zeros((128, WCOLS), np.float32)
    bpack = np.zeros((128, NBCOLS), np.float32)
    bpack[:, BC_NEG1] = -1.0

    def put(name, idx, arr, msz=None, prow=0):
        k, m = arr.shape
        base = WOFF[name] + idx * (msz if msz is not None else m)
        wpack[prow:prow + k, base:base + m] = arr

    W1 = np.asarray(vae_W1, np.float32)  # [480, 256] rows already in vae_hist order
    for c in range(4):
        chunk = W1[120 * c:120 * c + 120]
        if c == 3:
            chunk = np.vstack([chunk, (np.asarray(vae_b1) + 1.0)[None]])
        put("w1", c, chunk, msz=256)
    # [Wv | Wz] order so the evac lands [v_pred(3) | z_H(32)] at inpB[0:35]
    Wzv = np.concatenate([vae_Wv, vae_Wz], axis=1).astype(np.float32)  # [256,35]
    put("wzv", 0, Wzv[0:128], msz=35)
    put("wzv", 1, Wzv[128:256], msz=35)
    bpack[0:35, BC_ZV] = np.concatenate([vae_bv, vae_bz]) - Wzv.sum(0)

    put("ae1", 0, np.vstack([ae_W1, (np.asarray(ae_b1) + 1.0)[None]]))
    put("ae2", 0, np.asarray(ae_W2, np.float32))
    bpack[0:32, BC_ZE] = np.asarray(ae_b2) - np.asarray(ae_W2).sum(0)

    put("g1", 0, np.vstack([gate_W1, (np.asarray(gate_b1) + 1.0)[None]]), prow=64)
    G2 = np.asarray(gate_W2, np.float32)  # [64,5]
    put("g2", 0, G2)
    bg2 = np.asarray(gate_b2) - G2.sum(0)  # [5]
    bpack[0:5, BC_G2] = bg2
    g2r1 = np.zeros((64, 128), np.float32)
    for e in range(4):
        g2r1[:, 32 * e:32 * e + 29] = G2[:, e:e + 1]
        bpack[32 * e:32 * e + 29, BC_G2R] = bg2[e]
    put("g2r1", 0, g2r1)
    g2r2 = np.repeat(G2[:, 4:5], 29, axis=1)
    put("g2r2", 0, g2r2)
    bpack[0:29, BC_G2R4] = bg2[4]

    for e in range(5):
        W1e = np.asarray(eW1[e], np.float32)  # [166,128]
        put("e1a", e, W1e[0:99], msz=128)
        e1b = np.zeros((97, 128), np.float32)
        e1b[0:35] = W1e[99:134]      # v_pred(3) + z_H(32) weight rows
        e1b[64:96] = W1e[134:166]    # z_E rows
        e1b[96] = np.asarray(eb1[e]) + 1.0
        put("e1b", e, e1b, msz=128)
        W2e = np.asarray(eW2[e], np.float32)
        c2 = np.asarray(eb2[e]) - W2e.sum(0)
        bpack[0:128, BC_C2 + e] = c2
        bpack[0:128, BC_C2P1 + e] = c2 + 1.0
        put("e2", e, W2e, msz=128)
        W3e = np.asarray(eW3[e], np.float32)
        W3p = np.zeros((128, 32), np.float32)
        W3p[:, 0:29] = W3e
        put("e3", e, W3p, msz=32)
        b3e = np.asarray(eb3[e]) - W3e.sum(0)  # [29]
        if e < 4:
            bpack[32 * e:32 * e + 29, BC_B3] = b3e
        else:
            bpack[0:29, BC_B34] = b3e
    msum = np.zeros((128, 29), np.float32)
    for e in range(4):
        msum[32 * e:32 * e + 29] = np.eye(29)
    put("msum", 0, msum)
    put("i29", 0, np.eye(29, dtype=np.float32))

    wpack16 = wpack.astype(np.float16)
    in_maps = []
    for c in range(n_cores):
        sl = slice(c * n_rows, (c + 1) * n_rows)
        in_maps.append({
            "xT": np.ascontiguousarray(xT[:, sl]),
            "wpack": wpack16,
            "bpack": bpack,
        })
    return in_maps


# ----------------------------------------------------------------- entry

_NC_CACHE = {}


def _get_program(n_rows=N_CORE, num_devices=N_CORES):
    key = (n_rows, num_devices)
    if key not in _NC_CACHE:
        _NC_CACHE[key] = build_program(n_rows, num_devices)
    return _NC_CACHE[key]


def kernel(**inputs):
    from concourse.bass_utils import run_bass_kernel_spmd

    nc = _get_program()
    in_maps = prep_inputs(**inputs)
    res = run_bass_kernel_spmd(nc, in_maps, core_ids=list(range(N_CORES)))
    out = np.empty((N_FULL, 29), np.float32)
    for c in range(N_CORES):
        num = res.results[c]["out_fm"]            # [29, n] unnormalized
        gl = res.results[c]["gl_fm"].astype(np.float32)  # [5, n]
        den = np.exp(gl).sum(axis=0)              # softmax denominator
        out[c * N_CORE:(c + 1) * N_CORE] = (num / den[None, :]).T
    return out
